# revision 35
# baseline (speedup 1.0000x reference)
"""Distributed Trainium2 Bass kernel: masked (upper-triangular) attention.

reference (L=4096, D=1024, fp32):
    Q = x @ Wq + bq ; K = z @ Wk + bk ; V = z @ Wv + bv
    S = Q @ K.T ; S[row > col] = -inf
    out = softmax(S / sqrt(D)) @ V

Strategy (8 NeuronCores, one TRN2 chip, SPMD):
  - Query rows dealt round-robin: core c owns rows {r : r % 8 == c}. This
    makes the causal (keep col >= row) footprint IDENTICAL on every core:
    query chunk m (128 local rows = global rows c+8*(128m..)) attends key
    tile t (512 keys) iff 2m <= t -> a uniform static 20-unit schedule that
    skips ~44% of the S/PV work with no per-core addressing.
  - K/V projections sharded over contiguous z blocks (512/core), AllGathered
    in bf16 into Shared-address-space DRAM (K^T as [d,keys], V natural).
  - S computed in [q, k] orientation (Q^T chunk stationary, K^T tile moving
    512-wide); exp on scalar engine emits row-sums via accum_out; P^T for
    the PV matmul obtained with PE transposes of the 128x128 es chunks.
  - Only the two near-diagonal tiles per chunk need masks: two constant
    [128,512] additive (-50) masks built once from an iota + core id.
  - Matmuls in bf16 with fp32 PSUM accumulation.
"""

import math

import ml_dtypes
import numpy as np

BF16_NP = ml_dtypes.bfloat16

import concourse.mybir as mybir
import concourse.tile as tile
from concourse import bacc
from concourse.bass_utils import run_bass_kernel_spmd

F32 = mybir.dt.float32
BF16 = mybir.dt.bfloat16
AF = mybir.ActivationFunctionType
OP = mybir.AluOpType
P = 128
NCORES = 8

L = 4096
D = 1024


def build_graph(Ldim=L, Ddim=D):
    nc = bacc.Bacc("TRN2", target_bir_lowering=False, debug=False, num_devices=NCORES)
    ROWS = Ldim // NCORES        # query rows per core
    MB = ROWS // P               # 128-row query chunks per core (4)
    ZB = ROWS // P               # z-shard 128-row blocks (4)
    SW = ROWS                    # key-tile width == z-shard width (512)
    JT = SW // P                 # 128-key subtiles per key tile (4)
    NT = NCORES                  # one key tile per shard
    IO = Ddim // P               # contraction chunks (8)
    AO = Ddim // P               # d_attn 128-blocks (8)
    VH = Ddim // 512             # 512-wide value column halves (2)
    scale = 1.0 / math.sqrt(Ddim)
    # units (t, m) with 2m <= t; unit index = UOFF[t] + m
    UCNT = [t // 2 + 1 for t in range(NT)]
    UOFF = [sum(UCNT[:t]) for t in range(NT)]
    NU = sum(UCNT)               # 20

    x_ext = nc.declare_dram_parameter("x", [P, MB, Ddim], BF16, isOutput=False)
    z_ext = nc.declare_dram_parameter("z", [P, ZB, Ddim], BF16, isOutput=False)
    wq_ext = nc.declare_dram_parameter("Wq", [Ddim, Ddim], BF16, isOutput=False)
    wk_ext = nc.declare_dram_parameter("Wk", [Ddim, Ddim], BF16, isOutput=False)
    wv_ext = nc.declare_dram_parameter("Wv", [Ddim, Ddim], BF16, isOutput=False)
    bq_ext = nc.declare_dram_parameter("bq", [Ddim], F32, isOutput=False)
    bk_ext = nc.declare_dram_parameter("bk", [Ddim], F32, isOutput=False)
    bv_ext = nc.declare_dram_parameter("bv", [Ddim], F32, isOutput=False)
    cval_ext = nc.declare_dram_parameter("cval", [1], F32, isOutput=False)
    out_ext = nc.declare_dram_parameter("out", [ROWS, Ddim], F32, isOutput=True)

    ident_d = nc.inline_tensor(np.eye(P, dtype=np.float32), name="ident_c")
    identb_d = nc.inline_tensor(np.eye(P, dtype=np.float32), name="identb_c")

    with tile.TileContext(nc) as tc:
        with tc.tile_pool(name="const", bufs=1) as constp, \
             tc.tile_pool(name="persist", bufs=1) as persist, \
             tc.tile_pool(name="dram", bufs=1, space="DRAM") as dram:
            identf = constp.tile([P, P], F32)
            nc.scalar.dma_start(out=identf[:], in_=identb_d.ap())
            identb = constp.tile([P, P], BF16)
            nc.vector.tensor_copy(identb[:], identf[:])
            bvb = constp.tile([P, Ddim], F32)
            nc.scalar.dma_start(out=bvb[:], in_=bv_ext[:].partition_broadcast(P))
            bqs = constp.tile([P, AO], F32)
            nc.scalar.dma_start(out=bqs[:], in_=bq_ext[:].rearrange("(ao p) -> p ao", p=P))
            bks = constp.tile([P, AO], F32)
            nc.scalar.dma_start(out=bks[:], in_=bk_ext[:].rearrange("(ao p) -> p ao", p=P))
            cvb = constp.tile([P, 1], F32)
            nc.scalar.dma_start(out=cvb[:], in_=cval_ext[:].partition_broadcast(P))

            QT = persist.tile([P, AO, ROWS], BF16)
            KH = 1                       # key splits (1: single K AllGather)
            KW = AO * (ROWS // KH)       # flat K width per partition per half
            VW = ZB * Ddim               # flat V width per partition
            kt_bds = [dram.tile([P, AO, ROWS // KH], BF16, name=f"kt_bd{h}")
                      for h in range(KH)]
            v_bds = [dram.tile([P, VW // VH], BF16, name=f"v_bd{vh}") for vh in range(VH)]
            kt_gds = [dram.tile([NCORES, P, AO, ROWS // KH], BF16, name=f"kt_gd{h}",
                                addr_space="Shared") for h in range(KH)]
            v_gds = [dram.tile([NCORES, P, VW // VH], BF16, name=f"v_gd{vh}",
                               addr_space="Shared") for vh in range(VH)]

            # additive pre-softmax masks for the two near-diagonal tiles of
            # each query chunk: with r = c + 8i + 1024m, keys k = 512t + f:
            #   t == 2m  : keep iff f - 8i - c >= 0        (maskA)
            #   t == 2m+1: keep iff f - 8i - c + 512 >= 0  (maskB)
            maskA = persist.tile([P, SW], F32)
            maskB = persist.tile([P, SW], F32)
            with tc.tile_pool(name="iop", bufs=1) as iop:
                iof = iop.tile([P, SW], F32)
                nc.gpsimd.iota(iof[:], pattern=[[1, SW]], base=0,
                               channel_multiplier=-8,
                               allow_small_or_imprecise_dtypes=True)
                tA = iop.tile([P, SW], F32)
                nc.vector.tensor_scalar(tA[:], iof[:], cvb[:], None, OP.subtract)
                mkA = iop.tile([P, SW], F32)
                nc.vector.tensor_scalar(mkA[:], tA[:], 0.0, None, OP.is_ge)
                nc.vector.tensor_scalar(maskA[:], mkA[:], 1.0, 50.0, OP.subtract, OP.mult)
                tB = iop.tile([P, SW], F32)
                nc.vector.tensor_scalar(tB[:], tA[:], 512.0, None, OP.add)
                mkB = iop.tile([P, SW], F32)
                nc.vector.tensor_scalar(mkB[:], tB[:], 0.0, None, OP.is_ge)
                nc.vector.tensor_scalar(maskB[:], mkB[:], 1.0, 50.0, OP.subtract, OP.mult)

            # ------- Phase 1+2: projections of own shards; K/V AllGathered -------
            # Inputs arrive pre-cast to bf16 from the host: weights DMA straight
            # into their SBUF tiles (no staging/cast), transposes run in bf16.
            with tc.tile_pool(name="inp", bufs=1) as inp, \
                 tc.tile_pool(name="wkv", bufs=1) as wp, \
                 tc.tile_pool(name="zp", bufs=1) as zp, \
                 tc.tile_pool(name="tpp", bufs=2, space="PSUM") as tpp, \
                 tc.tile_pool(name="pp", bufs=2, space="PSUM") as pp:
                zsb = inp.tile([P, ZB, Ddim], BF16)
                nc.sync.dma_start(out=zsb[:], in_=z_ext[:])
                xsb = inp.tile([P, MB, Ddim], BF16)
                nc.sync.dma_start(out=xsb[:], in_=x_ext[:])
                wk = wp.tile([P, IO, Ddim], BF16)
                wv = wp.tile([P, IO, Ddim], BF16)
                wq = wp.tile([P, IO, Ddim], BF16)
                nc.scalar.dma_start(out=wk[:], in_=wk_ext[:].rearrange("(io p) d -> p io d", p=P))
                zT = zp.tile([P, IO, ROWS], BF16)
                for io in range(IO):
                    for nb in range(ZB):
                        tp = tpp.tile([P, P], BF16, tag="tp", name=f"tp_{nb}_{io}")
                        nc.tensor.transpose(tp[:], zsb[:, nb, io * P:(io + 1) * P], identb[:])
                        nc.vector.tensor_copy(zT[:, io, nb * P:(nb + 1) * P], tp[:])

                KTs = inp.tile([P, AO, ROWS], BF16)
                for ao in range(AO):
                    kp = pp.tile([P, ROWS], F32, tag="kp", name=f"kp_{ao}")
                    for io in range(IO):
                        nc.tensor.matmul(kp[:], wk[:, io, ao * P:(ao + 1) * P], zT[:, io, :],
                                         start=(io == 0), stop=(io == IO - 1))
                    nc.vector.tensor_scalar(KTs[:, ao, :], kp[:], bks[:, ao:ao + 1], None, OP.add)
                KHW = ROWS // KH
                for h in range(KH):
                    nc.sync.dma_start(out=kt_bds[h][:], in_=KTs[:, :, h * KHW:(h + 1) * KHW])
                    nc.gpsimd.collective_compute(
                        "AllGather", OP.bypass, replica_groups=[list(range(NCORES))],
                        ins=[kt_bds[h][:].opt()], outs=[kt_gds[h][:].opt()])

                # V next: its AllGathers queue on the CC engine right behind K
                nc.scalar.dma_start(out=wv[:], in_=wv_ext[:].rearrange("(io p) d -> p io d", p=P))
                Vs = inp.tile([P, VH, ZB, 512], BF16)
                for nb in range(ZB):
                    vp = pp.tile([P, Ddim], F32, tag="vp", name=f"vp_{nb}", bufs=1)
                    for io in range(IO):
                        for vh in range(VH):
                            nc.tensor.matmul(vp[:, vh * 512:(vh + 1) * 512],
                                             zT[:, io, nb * P:(nb + 1) * P],
                                             wv[:, io, vh * 512:(vh + 1) * 512],
                                             start=(io == 0), stop=(io == IO - 1))
                    for vh in range(VH):
                        nc.vector.tensor_tensor(Vs[:, vh, nb, :], vp[:, vh * 512:(vh + 1) * 512],
                                                bvb[:, vh * 512:(vh + 1) * 512], OP.add)
                for vh in range(VH):
                    nc.sync.dma_start(out=v_bds[vh][:], in_=Vs[:, vh])
                    nc.gpsimd.collective_compute(
                        "AllGather", OP.bypass, replica_groups=[list(range(NCORES))],
                        ins=[v_bds[vh][:].opt()], outs=[v_gds[vh][:].opt()])

                # Q^T projection (overlaps the K/V AllGathers)
                nc.scalar.dma_start(out=wq[:], in_=wq_ext[:].rearrange("(io p) d -> p io d", p=P))
                xT = zp.tile([P, IO, ROWS], BF16)
                for io in range(IO):
                    for mb in range(MB):
                        tq = tpp.tile([P, P], BF16, tag="tp", name=f"tq_{mb}_{io}")
                        nc.tensor.transpose(tq[:], xsb[:, mb, io * P:(io + 1) * P], identb[:])
                        nc.vector.tensor_copy(xT[:, io, mb * P:(mb + 1) * P], tq[:])
                for ao in range(AO):
                    qp = pp.tile([P, ROWS], F32, tag="kp", name=f"qp_{ao}")
                    for io in range(IO):
                        nc.tensor.matmul(qp[:], wq[:, io, ao * P:(ao + 1) * P], xT[:, io, :],
                                         start=(io == 0), stop=(io == IO - 1))
                    # fold the softmax 1/sqrt(D) into Q^T
                    nc.vector.tensor_scalar(QT[:, ao, :], qp[:], bqs[:, ao:ao + 1], float(scale),
                                            OP.add, OP.mult)

            # ---------------- Phase 3: attention ----------------
            esT = persist.tile([P, NU, JT, P], BF16)     # P^T chunks for PV
            lacc = persist.tile([P, MB], F32)            # softmax denominators
            acc = persist.tile([P, MB, Ddim], F32)       # normalized output staging
            dmae = (nc.sync, nc.scalar)

            # S pass: S[q,k] = Q^T-chunk (stationary) x K^T half-tile (moving);
            # two sub-passes, one per gathered key-half so compute starts
            # right after the first K AllGather lands. Exp on scalar emits
            # row-sums via accum_out; PE transposes yield the P^T chunks for
            # PV, enqueued one unit behind so tensor never waits on the exp.
            KHW = SW // KH
            JH = JT // KH                # 128-key chunks per half (2)
            with tc.tile_pool(name="ktp", bufs=5) as ktp, \
                 tc.tile_pool(name="esp", bufs=4) as esp, \
                 tc.tile_pool(name="lpps", bufs=4) as lpps, \
                 tc.tile_pool(name="spp", bufs=3, space="PSUM") as spp, \
                 tc.tile_pool(name="tp2", bufs=2, space="PSUM") as tp2:
                pend = []

                def flush_pend():
                    for (pes, pu, ph) in pend:
                        for j in range(JH):
                            kc = ph * JH + j
                            tp = tp2.tile([P, P], BF16, tag="tp2", name=f"tp2_{pu}_{kc}")
                            nc.tensor.transpose(tp[:], pes[:, j * P:(j + 1) * P], identb[:])
                            if kc % 2 == 0:
                                nc.scalar.activation(esT[:, pu, kc, :], tp[:], AF.Copy)
                            else:
                                nc.vector.tensor_copy(esT[:, pu, kc, :], tp[:])
                    pend.clear()

                for h in range(KH):
                    for t in range(NT):
                        ktt = ktp.tile([P, AO, KHW], BF16, tag="ktt", name=f"ktt_{h}_{t}")
                        if t == 0:
                            # split the first tile's load so the S pass starts
                            # on ao-chunk 0 without waiting for the full tile
                            nc.sync.dma_start(out=ktt[:, 0:2, :], in_=kt_gds[h][t][:, 0:2, :])
                            nc.sync.dma_start(out=ktt[:, 2:AO, :], in_=kt_gds[h][t][:, 2:AO, :])
                        else:
                            nc.sync.dma_start(out=ktt[:], in_=kt_gds[h][t])
                        for m in range(t // 2 + 1):
                            u = UOFF[t] + m
                            sp = spp.tile([P, KHW], F32, tag="sp", name=f"sp_{u}_{h}")
                            for ao in range(AO):
                                nc.tensor.matmul(sp[:], QT[:, ao, m * P:(m + 1) * P],
                                                 ktt[:, ao, :], start=(ao == 0),
                                                 stop=(ao == AO - 1))
                            flush_pend()
                            if t == 2 * m:
                                nc.vector.tensor_tensor(sp[:], sp[:],
                                                        maskA[:, h * KHW:(h + 1) * KHW],
                                                        OP.add)
                            elif t == 2 * m + 1:
                                nc.vector.tensor_tensor(sp[:], sp[:],
                                                        maskB[:, h * KHW:(h + 1) * KHW],
                                                        OP.add)
                            es = esp.tile([P, KHW], BF16, tag="es", name=f"es_{u}_{h}")
                            lp = lpps.tile([P, 1], F32, tag="lp", name=f"lp_{u}_{h}")
                            nc.scalar.activation(es[:], sp[:], AF.Exp, accum_out=lp[:])
                            if t == 2 * m and h == 0:
                                nc.vector.tensor_copy(lacc[:, m:m + 1], lp[:])
                            else:
                                nc.vector.tensor_tensor(lacc[:, m:m + 1], lacc[:, m:m + 1],
                                                        lp[:], OP.add)
                            pend.append((es, u, h))
                flush_pend()

            # PV pass per value-half, tiles descending so the deepest chunks
            # start immediately after the S pass; psum per query chunk. Each
            # chunk is normalized (and on the second half, written out) as
            # soon as its accumulation stops, spreading the output DMAs.
            oview = out_ext[:].rearrange("(mb p) v -> p mb v", p=P)
            with tc.tile_pool(name="vtp", bufs=8) as vtp, \
                 tc.tile_pool(name="recp", bufs=1) as recp, \
                 tc.tile_pool(name="pvp", bufs=1, space="PSUM") as pvp:
                rec = recp.tile([P, MB], F32)
                nc.vector.reciprocal(rec[:], lacc[:])
                for vh in range(VH):
                    pvs = [pvp.tile([P, 512], F32, tag=f"pv{m}", name=f"pv{vh}_{m}")
                           for m in range(MB)]
                    for t in range(NT - 1, -1, -1):
                        vtt = vtp.tile([P, JT, 512], BF16, tag="vtt", name=f"vtt_{vh}_{t}")
                        nc.gpsimd.dma_start(out=vtt[:], in_=v_gds[vh][t])
                        for m in range(t // 2 + 1):
                            u = UOFF[t] + m
                            for kc in range(JT):
                                nc.tensor.matmul(pvs[m][:], esT[:, u, kc, :],
                                                 vtt[:, kc, :],
                                                 start=(t == NT - 1 and kc == 0),
                                                 stop=(t == 2 * m and kc == JT - 1))
                        if t % 2 == 0:
                            m = t // 2
                            nc.scalar.activation(acc[:, m, vh * 512:(vh + 1) * 512],
                                                 pvs[m][:], AF.Copy, scale=rec[:, m:m + 1])
                            nc.sync.dma_start(out=oview[:, m, vh * 512:(vh + 1) * 512],
                                              in_=acc[:, m, vh * 512:(vh + 1) * 512])
    nc.compile()
    return nc


_GRAPH_CACHE = {}


def _get_graph(Ldim=L, Ddim=D):
    key = (Ldim, Ddim)
    if key not in _GRAPH_CACHE:
        _GRAPH_CACHE[key] = build_graph(Ldim, Ddim)
    return _GRAPH_CACHE[key]


def kernel(x, z, Wq, bq, Wk, bk, Wv, bv):
    x = np.ascontiguousarray(np.asarray(x, dtype=np.float32)).astype(BF16_NP)
    z = np.ascontiguousarray(np.asarray(z, dtype=np.float32)).astype(BF16_NP)
    Ldim, Ddim = x.shape
    NPART = P
    nc = _get_graph(Ldim, Ddim)
    ROWS = Ldim // NCORES
    common = {
        "Wq": np.ascontiguousarray(np.asarray(Wq, np.float32).astype(BF16_NP)),
        "bq": np.ascontiguousarray(np.asarray(bq, np.float32)),
        "Wk": np.ascontiguousarray(np.asarray(Wk, np.float32).astype(BF16_NP)),
        "bk": np.ascontiguousarray(np.asarray(bk, np.float32)),
        "Wv": np.ascontiguousarray(np.asarray(Wv, np.float32).astype(BF16_NP)),
        "bv": np.ascontiguousarray(np.asarray(bv, np.float32)),
    }
    in_maps = []
    for c in range(NCORES):
        m = dict(common)
        xc = x[c::NCORES]                      # interleaved query rows
        zc = z[ROWS * c:ROWS * (c + 1)]        # contiguous key rows
        m["x"] = np.ascontiguousarray(
            xc.reshape(ROWS // NPART, NPART, Ddim).transpose(1, 0, 2))
        m["z"] = np.ascontiguousarray(
            zc.reshape(ROWS // NPART, NPART, Ddim).transpose(1, 0, 2))
        m["cval"] = np.array([c], dtype=np.float32)
        in_maps.append(m)
    try:
        res = run_bass_kernel_spmd(nc, in_maps, core_ids=list(range(NCORES)))
    except Exception:
        # transient NRT device hiccups have been observed; one retry
        res = run_bass_kernel_spmd(nc, in_maps, core_ids=list(range(NCORES)))
    out = np.empty((Ldim, Ddim), dtype=np.float32)
    for c in range(NCORES):
        out[c::NCORES] = res.results[c]["out"]
    return out


# revision 36
# speedup vs baseline: 1.2774x; 1.2774x over previous
"""Distributed Trainium2 Bass kernel: masked (upper-triangular) attention.

reference (L=4096, D=1024, fp32):
    Q = x @ Wq + bq ; K = z @ Wk + bk ; V = z @ Wv + bv
    S = Q @ K.T ; S[row > col] = -inf
    out = softmax(S / sqrt(D)) @ V

Strategy (8 NeuronCores, one TRN2 chip, SPMD):
  - Query rows dealt round-robin: core c owns rows {r : r % 8 == c}. This
    makes the causal (keep col >= row) footprint IDENTICAL on every core:
    query chunk m (128 local rows = global rows c+8*(128m..)) attends key
    tile t (512 keys) iff 2m <= t -> a uniform static 20-unit schedule that
    skips ~44% of the S/PV work with no per-core addressing.
  - K/V projections sharded over contiguous z blocks (512/core), AllGathered
    in bf16 into Shared-address-space DRAM (K^T as [d,keys], V natural).
  - S computed in [q, k] orientation (Q^T chunk stationary, K^T tile moving
    512-wide); exp on scalar engine emits row-sums via accum_out; P^T for
    the PV matmul obtained with PE transposes of the 128x128 es chunks.
  - Only the two near-diagonal tiles per chunk need masks: two constant
    [128,512] additive (-50) masks built once from an iota + core id.
  - Matmuls in bf16 with fp32 PSUM accumulation.
"""

import math

import ml_dtypes
import numpy as np

BF16_NP = ml_dtypes.bfloat16

import concourse.mybir as mybir
import concourse.tile as tile
from concourse import bacc
from concourse.bass_utils import run_bass_kernel_spmd

F32 = mybir.dt.float32
BF16 = mybir.dt.bfloat16
AF = mybir.ActivationFunctionType
OP = mybir.AluOpType
P = 128
NCORES = 8

L = 4096
D = 1024


def build_graph(Ldim=L, Ddim=D):
    nc = bacc.Bacc("TRN2", target_bir_lowering=False, debug=False, num_devices=NCORES)
    ROWS = Ldim // NCORES        # query rows per core
    MB = ROWS // P               # 128-row query chunks per core (4)
    ZB = ROWS // P               # z-shard 128-row blocks (4)
    SW = ROWS                    # key-tile width == z-shard width (512)
    JT = SW // P                 # 128-key subtiles per key tile (4)
    NT = NCORES                  # one key tile per shard
    IO = Ddim // P               # contraction chunks (8)
    AO = Ddim // P               # d_attn 128-blocks (8)
    VH = Ddim // 512             # 512-wide value column halves (2)
    scale = 1.0 / math.sqrt(Ddim)
    # units (t, m) with 2m <= t; unit index = UOFF[t] + m
    UCNT = [t // 2 + 1 for t in range(NT)]
    UOFF = [sum(UCNT[:t]) for t in range(NT)]
    NU = sum(UCNT)               # 20

    x_ext = nc.declare_dram_parameter("x", [P, MB, Ddim], BF16, isOutput=False)
    z_ext = nc.declare_dram_parameter("z", [P, ZB, Ddim], BF16, isOutput=False)
    wq_ext = nc.declare_dram_parameter("Wq", [Ddim, Ddim], BF16, isOutput=False)
    wk_ext = nc.declare_dram_parameter("Wk", [Ddim, Ddim], BF16, isOutput=False)
    wv_ext = nc.declare_dram_parameter("Wv", [Ddim, Ddim], BF16, isOutput=False)
    bq_ext = nc.declare_dram_parameter("bq", [Ddim], F32, isOutput=False)
    bk_ext = nc.declare_dram_parameter("bk", [Ddim], F32, isOutput=False)
    bv_ext = nc.declare_dram_parameter("bv", [Ddim], F32, isOutput=False)
    cval_ext = nc.declare_dram_parameter("cval", [1], F32, isOutput=False)
    out_ext = nc.declare_dram_parameter("out", [ROWS, Ddim], F32, isOutput=True)

    ident_d = nc.inline_tensor(np.eye(P, dtype=np.float32), name="ident_c")
    identb_d = nc.inline_tensor(np.eye(P, dtype=np.float32), name="identb_c")

    with tile.TileContext(nc) as tc:
        with tc.tile_pool(name="const", bufs=1) as constp, \
             tc.tile_pool(name="persist", bufs=1) as persist, \
             tc.tile_pool(name="dram", bufs=1, space="DRAM") as dram:
            identf = constp.tile([P, P], F32)
            nc.scalar.dma_start(out=identf[:], in_=identb_d.ap())
            identb = constp.tile([P, P], BF16)
            nc.vector.tensor_copy(identb[:], identf[:])
            bvb = constp.tile([P, Ddim], F32)
            nc.scalar.dma_start(out=bvb[:], in_=bv_ext[:].partition_broadcast(P))
            bqs = constp.tile([P, AO], F32)
            nc.scalar.dma_start(out=bqs[:], in_=bq_ext[:].rearrange("(ao p) -> p ao", p=P))
            bks = constp.tile([P, AO], F32)
            nc.scalar.dma_start(out=bks[:], in_=bk_ext[:].rearrange("(ao p) -> p ao", p=P))
            cvb = constp.tile([P, 1], F32)
            nc.scalar.dma_start(out=cvb[:], in_=cval_ext[:].partition_broadcast(P))

            QT = persist.tile([P, AO, ROWS], BF16)
            KH = 1                       # key splits (1: single K AllGather)
            KW = AO * (ROWS // KH)       # flat K width per partition per half
            VW = ZB * Ddim               # flat V width per partition
            kt_bds = [dram.tile([P, AO, ROWS // KH], BF16, name=f"kt_bd{h}")
                      for h in range(KH)]
            v_bds = [dram.tile([P, VW // VH], BF16, name=f"v_bd{vh}") for vh in range(VH)]
            kt_gds = [dram.tile([NCORES, P, AO, ROWS // KH], BF16, name=f"kt_gd{h}",
                                addr_space="Shared") for h in range(KH)]
            v_gds = [dram.tile([NCORES, P, VW // VH], BF16, name=f"v_gd{vh}",
                               addr_space="Shared") for vh in range(VH)]

            # additive pre-softmax masks for the two near-diagonal tiles of
            # each query chunk: with r = c + 8i + 1024m, keys k = 512t + f:
            #   t == 2m  : keep iff f - 8i - c >= 0        (maskA)
            #   t == 2m+1: keep iff f - 8i - c + 512 >= 0  (maskB)
            maskA = persist.tile([P, SW], F32)
            maskB = persist.tile([P, SW], F32)
            with tc.tile_pool(name="iop", bufs=1) as iop:
                iof = iop.tile([P, SW], F32)
                nc.gpsimd.iota(iof[:], pattern=[[1, SW]], base=0,
                               channel_multiplier=-8,
                               allow_small_or_imprecise_dtypes=True)
                tA = iop.tile([P, SW], F32)
                nc.vector.tensor_scalar(tA[:], iof[:], cvb[:], None, OP.subtract)
                mkA = iop.tile([P, SW], F32)
                nc.vector.tensor_scalar(mkA[:], tA[:], 0.0, None, OP.is_ge)
                nc.vector.tensor_scalar(maskA[:], mkA[:], 1.0, 50.0, OP.subtract, OP.mult)
                tB = iop.tile([P, SW], F32)
                nc.vector.tensor_scalar(tB[:], tA[:], 512.0, None, OP.add)
                mkB = iop.tile([P, SW], F32)
                nc.vector.tensor_scalar(mkB[:], tB[:], 0.0, None, OP.is_ge)
                nc.vector.tensor_scalar(maskB[:], mkB[:], 1.0, 50.0, OP.subtract, OP.mult)

            # ------- Phase 1+2: projections of own shards; K/V AllGathered -------
            # Inputs arrive pre-cast to bf16 from the host: weights DMA straight
            # into their SBUF tiles (no staging/cast), transposes run in bf16.
            with tc.tile_pool(name="inp", bufs=1) as inp, \
                 tc.tile_pool(name="wkv", bufs=1) as wp, \
                 tc.tile_pool(name="zp", bufs=1) as zp, \
                 tc.tile_pool(name="tpp", bufs=2, space="PSUM") as tpp, \
                 tc.tile_pool(name="pp", bufs=2, space="PSUM") as pp:
                zsb = inp.tile([P, ZB, Ddim], BF16)
                nc.sync.dma_start(out=zsb[:], in_=z_ext[:])
                xsb = inp.tile([P, MB, Ddim], BF16)
                nc.sync.dma_start(out=xsb[:], in_=x_ext[:])
                wk = wp.tile([P, IO, Ddim], BF16)
                wv = wp.tile([P, IO, Ddim], BF16)
                wq = wp.tile([P, IO, Ddim], BF16)
                nc.scalar.dma_start(out=wk[:], in_=wk_ext[:].rearrange("(io p) d -> p io d", p=P))
                zT = zp.tile([P, IO, ROWS], BF16)
                for io in range(IO):
                    for nb in range(ZB):
                        tp = tpp.tile([P, P], BF16, tag="tp", name=f"tp_{nb}_{io}")
                        nc.tensor.transpose(tp[:], zsb[:, nb, io * P:(io + 1) * P], identb[:])
                        nc.vector.tensor_copy(zT[:, io, nb * P:(nb + 1) * P], tp[:])

                KTs = inp.tile([P, AO, ROWS], BF16)
                for ao in range(AO):
                    kp = pp.tile([P, ROWS], F32, tag="kp", name=f"kp_{ao}")
                    for io in range(IO):
                        nc.tensor.matmul(kp[:], wk[:, io, ao * P:(ao + 1) * P], zT[:, io, :],
                                         start=(io == 0), stop=(io == IO - 1))
                    nc.vector.tensor_scalar(KTs[:, ao, :], kp[:], bks[:, ao:ao + 1], None, OP.add)
                KHW = ROWS // KH
                for h in range(KH):
                    nc.sync.dma_start(out=kt_bds[h][:], in_=KTs[:, :, h * KHW:(h + 1) * KHW])
                    nc.gpsimd.collective_compute(
                        "AllGather", OP.bypass, replica_groups=[list(range(NCORES))],
                        ins=[kt_bds[h][:].opt()], outs=[kt_gds[h][:].opt()])

                # V next: its AllGathers queue on the CC engine right behind K
                nc.scalar.dma_start(out=wv[:], in_=wv_ext[:].rearrange("(io p) d -> p io d", p=P))
                Vs = inp.tile([P, VH, ZB, 512], BF16)
                for nb in range(ZB):
                    vp = pp.tile([P, Ddim], F32, tag="vp", name=f"vp_{nb}", bufs=1)
                    for io in range(IO):
                        for vh in range(VH):
                            nc.tensor.matmul(vp[:, vh * 512:(vh + 1) * 512],
                                             zT[:, io, nb * P:(nb + 1) * P],
                                             wv[:, io, vh * 512:(vh + 1) * 512],
                                             start=(io == 0), stop=(io == IO - 1))
                    for vh in range(VH):
                        nc.vector.tensor_tensor(Vs[:, vh, nb, :], vp[:, vh * 512:(vh + 1) * 512],
                                                bvb[:, vh * 512:(vh + 1) * 512], OP.add)
                for vh in range(VH):
                    nc.sync.dma_start(out=v_bds[vh][:], in_=Vs[:, vh])
                    nc.gpsimd.collective_compute(
                        "AllGather", OP.bypass, replica_groups=[list(range(NCORES))],
                        ins=[v_bds[vh][:].opt()], outs=[v_gds[vh][:].opt()])

                # Q^T projection (overlaps the K/V AllGathers)
                nc.scalar.dma_start(out=wq[:], in_=wq_ext[:].rearrange("(io p) d -> p io d", p=P))
                xT = zp.tile([P, IO, ROWS], BF16)
                for io in range(IO):
                    for mb in range(MB):
                        tq = tpp.tile([P, P], BF16, tag="tp", name=f"tq_{mb}_{io}")
                        nc.tensor.transpose(tq[:], xsb[:, mb, io * P:(io + 1) * P], identb[:])
                        nc.vector.tensor_copy(xT[:, io, mb * P:(mb + 1) * P], tq[:])
                for ao in range(AO):
                    qp = pp.tile([P, ROWS], F32, tag="kp", name=f"qp_{ao}")
                    for io in range(IO):
                        nc.tensor.matmul(qp[:], wq[:, io, ao * P:(ao + 1) * P], xT[:, io, :],
                                         start=(io == 0), stop=(io == IO - 1))
                    # fold the softmax 1/sqrt(D) into Q^T
                    nc.vector.tensor_scalar(QT[:, ao, :], qp[:], bqs[:, ao:ao + 1], float(scale),
                                            OP.add, OP.mult)

            # ---------------- Phase 3: attention ----------------
            esT = persist.tile([P, NU, JT, P], BF16)     # P^T chunks for PV
            lacc = persist.tile([P, MB], F32)            # softmax denominators
            acc = persist.tile([P, MB, Ddim], F32)       # normalized output staging
            dmae = (nc.sync, nc.scalar)

            # S pass runs tiles DESCENDING (thick-to-thin matches the
            # post-AllGather load ramp) and the PV pass for the first value
            # half is interleaved into the S tail: its tiles become ready in
            # exactly the order PV consumes them, and V0's AllGather lands
            # mid-S. vh0 normalizations are deferred past the interleave
            # (their reciprocal needs the full row-sum). Exp on scalar emits
            # row-sums via accum_out; PE transposes yield the P^T chunks,
            # enqueued one unit behind so tensor never waits on the exp.
            oview = out_ext[:].rearrange("(mb p) v -> p mb v", p=P)
            KHW = SW // KH
            JH = JT // KH                # 128-key chunks per half (2)
            with tc.tile_pool(name="ktp", bufs=5) as ktp, \
                 tc.tile_pool(name="esp", bufs=4) as esp, \
                 tc.tile_pool(name="lpps", bufs=4) as lpps, \
                 tc.tile_pool(name="vtp", bufs=8) as vtp, \
                 tc.tile_pool(name="recp", bufs=1) as recp, \
                 tc.tile_pool(name="spp", bufs=2, space="PSUM") as spp, \
                 tc.tile_pool(name="tp2", bufs=2, space="PSUM") as tp2, \
                 tc.tile_pool(name="pvp", bufs=1, space="PSUM") as pvp:
                pend = []

                def flush_pend():
                    for (pes, pu, ph) in pend:
                        for j in range(JH):
                            kc = ph * JH + j
                            tp = tp2.tile([P, P], BF16, tag="tp2", name=f"tp2_{pu}_{kc}")
                            nc.tensor.transpose(tp[:], pes[:, j * P:(j + 1) * P], identb[:])
                            if kc % 2 == 0:
                                nc.scalar.activation(esT[:, pu, kc, :], tp[:], AF.Copy)
                            else:
                                nc.vector.tensor_copy(esT[:, pu, kc, :], tp[:])
                    pend.clear()

                def emit_s(t, h=0):
                    ktt = ktp.tile([P, AO, KHW], BF16, tag="ktt", name=f"ktt_{h}_{t}")
                    if t == NT - 1:
                        # split the first tile's load so the S pass starts
                        # on ao-chunk 0 without waiting for the full tile
                        nc.sync.dma_start(out=ktt[:, 0:2, :], in_=kt_gds[h][t][:, 0:2, :])
                        nc.sync.dma_start(out=ktt[:, 2:AO, :], in_=kt_gds[h][t][:, 2:AO, :])
                    else:
                        nc.sync.dma_start(out=ktt[:], in_=kt_gds[h][t])
                    for m in range(t // 2 + 1):
                        u = UOFF[t] + m
                        sp = spp.tile([P, KHW], F32, tag="sp", name=f"sp_{u}_{h}")
                        for ao in range(AO):
                            nc.tensor.matmul(sp[:], QT[:, ao, m * P:(m + 1) * P],
                                             ktt[:, ao, :], start=(ao == 0),
                                             stop=(ao == AO - 1))
                        flush_pend()
                        if t == 2 * m:
                            nc.vector.tensor_tensor(sp[:], sp[:],
                                                    maskA[:, h * KHW:(h + 1) * KHW],
                                                    OP.add)
                        elif t == 2 * m + 1:
                            nc.vector.tensor_tensor(sp[:], sp[:],
                                                    maskB[:, h * KHW:(h + 1) * KHW],
                                                    OP.add)
                        es = esp.tile([P, KHW], BF16, tag="es", name=f"es_{u}_{h}")
                        lp = lpps.tile([P, 1], F32, tag="lp", name=f"lp_{u}_{h}")
                        nc.scalar.activation(es[:], sp[:], AF.Exp, accum_out=lp[:])
                        if t == NT - 1 and h == 0:
                            nc.vector.tensor_copy(lacc[:, m:m + 1], lp[:])
                        else:
                            nc.vector.tensor_tensor(lacc[:, m:m + 1], lacc[:, m:m + 1],
                                                    lp[:], OP.add)
                        pend.append((es, u, h))

                pv0s = [pvp.tile([P, 512], F32, tag=f"pv{m}", name=f"pv0_{m}")
                        for m in range(MB)]

                def emit_pv0(t):
                    flush_pend()
                    vtt = vtp.tile([P, JT, 512], BF16, tag="vtt", name=f"vtt_0_{t}")
                    nc.gpsimd.dma_start(out=vtt[:], in_=v_gds[0][t])
                    for m in range(t // 2 + 1):
                        u = UOFF[t] + m
                        for kc in range(JT):
                            nc.tensor.matmul(pv0s[m][:], esT[:, u, kc, :], vtt[:, kc, :],
                                             start=(t == NT - 1 and kc == 0),
                                             stop=(t == 2 * m and kc == JT - 1))

                # interleaved schedule: big S tiles first, PV-vh0 woven in
                for t in range(NT - 1, 2, -1):
                    emit_s(t)
                emit_pv0(7)
                emit_s(2)
                emit_pv0(6)
                emit_s(1)
                emit_pv0(5)
                emit_s(0)
                for t in range(4, -1, -1):
                    emit_pv0(t)
                flush_pend()

                # vh0 normalizations (deferred: need the complete row-sums)
                rec = recp.tile([P, MB], F32)
                nc.vector.reciprocal(rec[:], lacc[:])
                for m in range(MB):
                    nc.vector.tensor_scalar(acc[:, m, 0:512], pv0s[m][:],
                                            rec[:, m:m + 1], None, OP.mult)
                    nc.sync.dma_start(out=oview[:, m, 0:512], in_=acc[:, m, 0:512])

                # PV pass for the second value half, tiles descending;
                # per-chunk scalar normalize + output write at each stop
                pv1s = [pvp.tile([P, 512], F32, tag=f"pv{m}", name=f"pv1_{m}")
                        for m in range(MB)]
                for t in range(NT - 1, -1, -1):
                    vtt = vtp.tile([P, JT, 512], BF16, tag="vtt", name=f"vtt_1_{t}")
                    nc.gpsimd.dma_start(out=vtt[:], in_=v_gds[1][t])
                    for m in range(t // 2 + 1):
                        u = UOFF[t] + m
                        for kc in range(JT):
                            nc.tensor.matmul(pv1s[m][:], esT[:, u, kc, :], vtt[:, kc, :],
                                             start=(t == NT - 1 and kc == 0),
                                             stop=(t == 2 * m and kc == JT - 1))
                    if t % 2 == 0:
                        m = t // 2
                        nc.scalar.activation(acc[:, m, 512:1024], pv1s[m][:],
                                             AF.Copy, scale=rec[:, m:m + 1])
                        nc.sync.dma_start(out=oview[:, m, 512:1024],
                                          in_=acc[:, m, 512:1024])
    nc.compile()
    return nc


_GRAPH_CACHE = {}


def _get_graph(Ldim=L, Ddim=D):
    key = (Ldim, Ddim)
    if key not in _GRAPH_CACHE:
        _GRAPH_CACHE[key] = build_graph(Ldim, Ddim)
    return _GRAPH_CACHE[key]


def kernel(x, z, Wq, bq, Wk, bk, Wv, bv):
    x = np.ascontiguousarray(np.asarray(x, dtype=np.float32)).astype(BF16_NP)
    z = np.ascontiguousarray(np.asarray(z, dtype=np.float32)).astype(BF16_NP)
    Ldim, Ddim = x.shape
    NPART = P
    nc = _get_graph(Ldim, Ddim)
    ROWS = Ldim // NCORES
    common = {
        "Wq": np.ascontiguousarray(np.asarray(Wq, np.float32).astype(BF16_NP)),
        "bq": np.ascontiguousarray(np.asarray(bq, np.float32)),
        "Wk": np.ascontiguousarray(np.asarray(Wk, np.float32).astype(BF16_NP)),
        "bk": np.ascontiguousarray(np.asarray(bk, np.float32)),
        "Wv": np.ascontiguousarray(np.asarray(Wv, np.float32).astype(BF16_NP)),
        "bv": np.ascontiguousarray(np.asarray(bv, np.float32)),
    }
    in_maps = []
    for c in range(NCORES):
        m = dict(common)
        xc = x[c::NCORES]                      # interleaved query rows
        zc = z[ROWS * c:ROWS * (c + 1)]        # contiguous key rows
        m["x"] = np.ascontiguousarray(
            xc.reshape(ROWS // NPART, NPART, Ddim).transpose(1, 0, 2))
        m["z"] = np.ascontiguousarray(
            zc.reshape(ROWS // NPART, NPART, Ddim).transpose(1, 0, 2))
        m["cval"] = np.array([c], dtype=np.float32)
        in_maps.append(m)
    try:
        res = run_bass_kernel_spmd(nc, in_maps, core_ids=list(range(NCORES)))
    except Exception:
        # transient NRT device hiccups have been observed; one retry
        res = run_bass_kernel_spmd(nc, in_maps, core_ids=list(range(NCORES)))
    out = np.empty((Ldim, Ddim), dtype=np.float32)
    for c in range(NCORES):
        out[c::NCORES] = res.results[c]["out"]
    return out


# revision 37
# speedup vs baseline: 1.2933x; 1.0125x over previous
"""Distributed Trainium2 Bass kernel: masked (upper-triangular) attention.

reference (L=4096, D=1024, fp32):
    Q = x @ Wq + bq ; K = z @ Wk + bk ; V = z @ Wv + bv
    S = Q @ K.T ; S[row > col] = -inf
    out = softmax(S / sqrt(D)) @ V

Strategy (8 NeuronCores, one TRN2 chip, SPMD):
  - Query rows dealt round-robin: core c owns rows {r : r % 8 == c}. This
    makes the causal (keep col >= row) footprint IDENTICAL on every core:
    query chunk m (128 local rows = global rows c+8*(128m..)) attends key
    tile t (512 keys) iff 2m <= t -> a uniform static 20-unit schedule that
    skips ~44% of the S/PV work with no per-core addressing.
  - K/V projections sharded over contiguous z blocks (512/core), AllGathered
    in bf16 into Shared-address-space DRAM (K^T as [d,keys], V natural).
  - S computed in [q, k] orientation (Q^T chunk stationary, K^T tile moving
    512-wide); exp on scalar engine emits row-sums via accum_out; P^T for
    the PV matmul obtained with PE transposes of the 128x128 es chunks.
  - Only the two near-diagonal tiles per chunk need masks: two constant
    [128,512] additive (-50) masks built once from an iota + core id.
  - Matmuls in bf16 with fp32 PSUM accumulation.
"""

import math

import ml_dtypes
import numpy as np

BF16_NP = ml_dtypes.bfloat16

import concourse.mybir as mybir
import concourse.tile as tile
from concourse import bacc
from concourse.bass_utils import run_bass_kernel_spmd

F32 = mybir.dt.float32
BF16 = mybir.dt.bfloat16
AF = mybir.ActivationFunctionType
OP = mybir.AluOpType
P = 128
NCORES = 8

L = 4096
D = 1024


def build_graph(Ldim=L, Ddim=D):
    nc = bacc.Bacc("TRN2", target_bir_lowering=False, debug=False, num_devices=NCORES)
    ROWS = Ldim // NCORES        # query rows per core
    MB = ROWS // P               # 128-row query chunks per core (4)
    ZB = ROWS // P               # z-shard 128-row blocks (4)
    SW = ROWS                    # key-tile width == z-shard width (512)
    JT = SW // P                 # 128-key subtiles per key tile (4)
    NT = NCORES                  # one key tile per shard
    IO = Ddim // P               # contraction chunks (8)
    AO = Ddim // P               # d_attn 128-blocks (8)
    VH = Ddim // 512             # 512-wide value column halves (2)
    scale = 1.0 / math.sqrt(Ddim)
    # units (t, m) with 2m <= t; unit index = UOFF[t] + m
    UCNT = [t // 2 + 1 for t in range(NT)]
    UOFF = [sum(UCNT[:t]) for t in range(NT)]
    NU = sum(UCNT)               # 20

    x_ext = nc.declare_dram_parameter("x", [P, MB, Ddim], BF16, isOutput=False)
    z_ext = nc.declare_dram_parameter("z", [P, ZB, Ddim], BF16, isOutput=False)
    wq_ext = nc.declare_dram_parameter("Wq", [Ddim, Ddim], BF16, isOutput=False)
    wk_ext = nc.declare_dram_parameter("Wk", [Ddim, Ddim], BF16, isOutput=False)
    wv_ext = nc.declare_dram_parameter("Wv", [Ddim, Ddim], BF16, isOutput=False)
    bq_ext = nc.declare_dram_parameter("bq", [Ddim], F32, isOutput=False)
    bk_ext = nc.declare_dram_parameter("bk", [Ddim], F32, isOutput=False)
    bv_ext = nc.declare_dram_parameter("bv", [Ddim], F32, isOutput=False)
    cval_ext = nc.declare_dram_parameter("cval", [1], F32, isOutput=False)
    out_ext = nc.declare_dram_parameter("out", [ROWS, Ddim], F32, isOutput=True)

    ident_d = nc.inline_tensor(np.eye(P, dtype=np.float32), name="ident_c")
    identb_d = nc.inline_tensor(np.eye(P, dtype=np.float32), name="identb_c")

    with tile.TileContext(nc) as tc:
        with tc.tile_pool(name="const", bufs=1) as constp, \
             tc.tile_pool(name="persist", bufs=1) as persist, \
             tc.tile_pool(name="dram", bufs=1, space="DRAM") as dram:
            identf = constp.tile([P, P], F32)
            nc.scalar.dma_start(out=identf[:], in_=identb_d.ap())
            identb = constp.tile([P, P], BF16)
            nc.vector.tensor_copy(identb[:], identf[:])
            bvb = constp.tile([P, Ddim], F32)
            nc.scalar.dma_start(out=bvb[:], in_=bv_ext[:].partition_broadcast(P))
            bqs = constp.tile([P, AO], F32)
            nc.scalar.dma_start(out=bqs[:], in_=bq_ext[:].rearrange("(ao p) -> p ao", p=P))
            bks = constp.tile([P, AO], F32)
            nc.scalar.dma_start(out=bks[:], in_=bk_ext[:].rearrange("(ao p) -> p ao", p=P))
            cvb = constp.tile([P, 1], F32)
            nc.scalar.dma_start(out=cvb[:], in_=cval_ext[:].partition_broadcast(P))

            QT = persist.tile([P, AO, ROWS], BF16)
            KH = 1                       # key splits (1: single K AllGather)
            KW = AO * (ROWS // KH)       # flat K width per partition per half
            VW = ZB * Ddim               # flat V width per partition
            kt_bds = [dram.tile([P, AO, ROWS // KH], BF16, name=f"kt_bd{h}")
                      for h in range(KH)]
            v_bds = [dram.tile([P, VW // VH], BF16, name=f"v_bd{vh}") for vh in range(VH)]
            kt_gds = [dram.tile([NCORES, P, AO, ROWS // KH], BF16, name=f"kt_gd{h}",
                                addr_space="Shared") for h in range(KH)]
            v_gds = [dram.tile([NCORES, P, VW // VH], BF16, name=f"v_gd{vh}",
                               addr_space="Shared") for vh in range(VH)]

            # additive pre-softmax masks for the two near-diagonal tiles of
            # each query chunk: with r = c + 8i + 1024m, keys k = 512t + f:
            #   t == 2m  : keep iff f - 8i - c >= 0        (maskA)
            #   t == 2m+1: keep iff f - 8i - c + 512 >= 0  (maskB)
            maskA = persist.tile([P, SW], F32)
            maskB = persist.tile([P, SW], F32)
            with tc.tile_pool(name="iop", bufs=1) as iop:
                iof = iop.tile([P, SW], F32)
                nc.gpsimd.iota(iof[:], pattern=[[1, SW]], base=0,
                               channel_multiplier=-8,
                               allow_small_or_imprecise_dtypes=True)
                tA = iop.tile([P, SW], F32)
                nc.vector.tensor_scalar(tA[:], iof[:], cvb[:], None, OP.subtract)
                mkA = iop.tile([P, SW], F32)
                nc.vector.tensor_scalar(mkA[:], tA[:], 0.0, None, OP.is_ge)
                nc.vector.tensor_scalar(maskA[:], mkA[:], 1.0, 50.0, OP.subtract, OP.mult)
                tB = iop.tile([P, SW], F32)
                nc.vector.tensor_scalar(tB[:], tA[:], 512.0, None, OP.add)
                mkB = iop.tile([P, SW], F32)
                nc.vector.tensor_scalar(mkB[:], tB[:], 0.0, None, OP.is_ge)
                nc.vector.tensor_scalar(maskB[:], mkB[:], 1.0, 50.0, OP.subtract, OP.mult)

            # ------- Phase 1+2: projections of own shards; K/V AllGathered -------
            # Inputs arrive pre-cast to bf16 from the host: weights DMA straight
            # into their SBUF tiles (no staging/cast), transposes run in bf16.
            with tc.tile_pool(name="inp", bufs=1) as inp, \
                 tc.tile_pool(name="wkv", bufs=1) as wp, \
                 tc.tile_pool(name="zp", bufs=1) as zp, \
                 tc.tile_pool(name="tpp", bufs=2, space="PSUM") as tpp, \
                 tc.tile_pool(name="pp", bufs=2, space="PSUM") as pp:
                zsb = inp.tile([P, ZB, Ddim], BF16)
                nc.sync.dma_start(out=zsb[:], in_=z_ext[:])
                xsb = inp.tile([P, MB, Ddim], BF16)
                nc.sync.dma_start(out=xsb[:], in_=x_ext[:])
                wk = wp.tile([P, IO, Ddim], BF16)
                wv = wp.tile([P, IO, Ddim], BF16)
                wq = wp.tile([P, IO, Ddim], BF16)
                nc.scalar.dma_start(out=wk[:], in_=wk_ext[:].rearrange("(io p) d -> p io d", p=P))
                zT = zp.tile([P, IO, ROWS], BF16)
                for io in range(IO):
                    for nb in range(ZB):
                        tp = tpp.tile([P, P], BF16, tag="tp", name=f"tp_{nb}_{io}")
                        nc.tensor.transpose(tp[:], zsb[:, nb, io * P:(io + 1) * P], identb[:])
                        nc.vector.tensor_copy(zT[:, io, nb * P:(nb + 1) * P], tp[:])

                KTs = inp.tile([P, AO, ROWS], BF16)
                for ao in range(AO):
                    kp = pp.tile([P, ROWS], F32, tag="kp", name=f"kp_{ao}")
                    for io in range(IO):
                        nc.tensor.matmul(kp[:], wk[:, io, ao * P:(ao + 1) * P], zT[:, io, :],
                                         start=(io == 0), stop=(io == IO - 1))
                    nc.vector.tensor_scalar(KTs[:, ao, :], kp[:], bks[:, ao:ao + 1], None, OP.add)
                KHW = ROWS // KH
                for h in range(KH):
                    nc.sync.dma_start(out=kt_bds[h][:], in_=KTs[:, :, h * KHW:(h + 1) * KHW])
                    nc.gpsimd.collective_compute(
                        "AllGather", OP.bypass, replica_groups=[list(range(NCORES))],
                        ins=[kt_bds[h][:].opt()], outs=[kt_gds[h][:].opt()])

                # V next: its AllGathers queue on the CC engine right behind K
                nc.scalar.dma_start(out=wv[:], in_=wv_ext[:].rearrange("(io p) d -> p io d", p=P))
                Vs = inp.tile([P, VH, ZB, 512], BF16)
                for nb in range(ZB):
                    vp = pp.tile([P, Ddim], F32, tag="vp", name=f"vp_{nb}", bufs=1)
                    for io in range(IO):
                        for vh in range(VH):
                            nc.tensor.matmul(vp[:, vh * 512:(vh + 1) * 512],
                                             zT[:, io, nb * P:(nb + 1) * P],
                                             wv[:, io, vh * 512:(vh + 1) * 512],
                                             start=(io == 0), stop=(io == IO - 1))
                    for vh in range(VH):
                        nc.vector.tensor_tensor(Vs[:, vh, nb, :], vp[:, vh * 512:(vh + 1) * 512],
                                                bvb[:, vh * 512:(vh + 1) * 512], OP.add)
                for vh in range(VH):
                    nc.sync.dma_start(out=v_bds[vh][:], in_=Vs[:, vh])
                    nc.gpsimd.collective_compute(
                        "AllGather", OP.bypass, replica_groups=[list(range(NCORES))],
                        ins=[v_bds[vh][:].opt()], outs=[v_gds[vh][:].opt()])

                # Q^T projection (overlaps the K/V AllGathers)
                nc.scalar.dma_start(out=wq[:], in_=wq_ext[:].rearrange("(io p) d -> p io d", p=P))
                xT = zp.tile([P, IO, ROWS], BF16)
                for io in range(IO):
                    for mb in range(MB):
                        tq = tpp.tile([P, P], BF16, tag="tp", name=f"tq_{mb}_{io}")
                        nc.tensor.transpose(tq[:], xsb[:, mb, io * P:(io + 1) * P], identb[:])
                        nc.vector.tensor_copy(xT[:, io, mb * P:(mb + 1) * P], tq[:])
                for ao in range(AO):
                    qp = pp.tile([P, ROWS], F32, tag="kp", name=f"qp_{ao}")
                    for io in range(IO):
                        nc.tensor.matmul(qp[:], wq[:, io, ao * P:(ao + 1) * P], xT[:, io, :],
                                         start=(io == 0), stop=(io == IO - 1))
                    # fold the softmax 1/sqrt(D) into Q^T
                    nc.vector.tensor_scalar(QT[:, ao, :], qp[:], bqs[:, ao:ao + 1], float(scale),
                                            OP.add, OP.mult)

            # ---------------- Phase 3: attention ----------------
            esT = persist.tile([P, NU, JT, P], BF16)     # P^T chunks for PV
            lacc = persist.tile([P, MB], F32)            # softmax denominators
            acc = persist.tile([P, MB, Ddim], F32)       # normalized output staging
            dmae = (nc.sync, nc.scalar)

            # S pass: S[q,k] = Q^T-chunk (stationary) x K^T half-tile (moving);
            # two sub-passes, one per gathered key-half so compute starts
            # right after the first K AllGather lands. Exp on scalar emits
            # row-sums via accum_out; PE transposes yield the P^T chunks for
            # PV, enqueued one unit behind so tensor never waits on the exp.
            KHW = SW // KH
            JH = JT // KH                # 128-key chunks per half (2)
            with tc.tile_pool(name="ktp", bufs=5) as ktp, \
                 tc.tile_pool(name="esp", bufs=4) as esp, \
                 tc.tile_pool(name="lpps", bufs=4) as lpps, \
                 tc.tile_pool(name="spp", bufs=3, space="PSUM") as spp, \
                 tc.tile_pool(name="tp2", bufs=2, space="PSUM") as tp2:
                pend = []

                def flush_pend():
                    for (pes, pu, ph) in pend:
                        for j in range(JH):
                            kc = ph * JH + j
                            tp = tp2.tile([P, P], BF16, tag="tp2", name=f"tp2_{pu}_{kc}")
                            nc.tensor.transpose(tp[:], pes[:, j * P:(j + 1) * P], identb[:])
                            if kc % 2 == 0:
                                nc.scalar.activation(esT[:, pu, kc, :], tp[:], AF.Copy)
                            else:
                                nc.vector.tensor_copy(esT[:, pu, kc, :], tp[:])
                    pend.clear()

                for h in range(KH):
                    for t in range(NT):
                        ktt = ktp.tile([P, AO, KHW], BF16, tag="ktt", name=f"ktt_{h}_{t}")
                        if t == 0:
                            # split the first tile's load so the S pass starts
                            # on ao-chunk 0 without waiting for the full tile
                            nc.sync.dma_start(out=ktt[:, 0:2, :], in_=kt_gds[h][t][:, 0:2, :])
                            nc.sync.dma_start(out=ktt[:, 2:AO, :], in_=kt_gds[h][t][:, 2:AO, :])
                        else:
                            nc.sync.dma_start(out=ktt[:], in_=kt_gds[h][t])
                        for m in range(t // 2 + 1):
                            u = UOFF[t] + m
                            sp = spp.tile([P, KHW], F32, tag="sp", name=f"sp_{u}_{h}")
                            for ao in range(AO):
                                nc.tensor.matmul(sp[:], QT[:, ao, m * P:(m + 1) * P],
                                                 ktt[:, ao, :], start=(ao == 0),
                                                 stop=(ao == AO - 1))
                            flush_pend()
                            if t == 2 * m:
                                nc.vector.tensor_tensor(sp[:], sp[:],
                                                        maskA[:, h * KHW:(h + 1) * KHW],
                                                        OP.add)
                            elif t == 2 * m + 1:
                                nc.vector.tensor_tensor(sp[:], sp[:],
                                                        maskB[:, h * KHW:(h + 1) * KHW],
                                                        OP.add)
                            es = esp.tile([P, KHW], BF16, tag="es", name=f"es_{u}_{h}")
                            lp = lpps.tile([P, 1], F32, tag="lp", name=f"lp_{u}_{h}")
                            nc.scalar.activation(es[:], sp[:], AF.Exp, accum_out=lp[:])
                            if t == 2 * m and h == 0:
                                nc.vector.tensor_copy(lacc[:, m:m + 1], lp[:])
                            else:
                                nc.vector.tensor_tensor(lacc[:, m:m + 1], lacc[:, m:m + 1],
                                                        lp[:], OP.add)
                            pend.append((es, u, h))
                flush_pend()

            # PV pass per value-half, tiles descending so the deepest chunks
            # start immediately after the S pass; psum per query chunk. Each
            # chunk is normalized (and on the second half, written out) as
            # soon as its accumulation stops, spreading the output DMAs.
            oview = out_ext[:].rearrange("(mb p) v -> p mb v", p=P)
            with tc.tile_pool(name="vtp", bufs=8) as vtp, \
                 tc.tile_pool(name="recp", bufs=1) as recp, \
                 tc.tile_pool(name="pvp", bufs=1, space="PSUM") as pvp:
                rec = recp.tile([P, MB], F32)
                nc.vector.reciprocal(rec[:], lacc[:])
                for vh in range(VH):
                    pvs = [pvp.tile([P, 512], F32, tag=f"pv{m}", name=f"pv{vh}_{m}")
                           for m in range(MB)]
                    for t in range(NT - 1, -1, -1):
                        vtt = vtp.tile([P, JT, 512], BF16, tag="vtt", name=f"vtt_{vh}_{t}")
                        nc.gpsimd.dma_start(out=vtt[:], in_=v_gds[vh][t])
                        for m in range(t // 2 + 1):
                            u = UOFF[t] + m
                            for kc in range(JT):
                                nc.tensor.matmul(pvs[m][:], esT[:, u, kc, :],
                                                 vtt[:, kc, :],
                                                 start=(t == NT - 1 and kc == 0),
                                                 stop=(t == 2 * m and kc == JT - 1))
                        if t % 2 == 0:
                            m = t // 2
                            nc.scalar.activation(acc[:, m, vh * 512:(vh + 1) * 512],
                                                 pvs[m][:], AF.Copy, scale=rec[:, m:m + 1])
                            nc.sync.dma_start(out=oview[:, m, vh * 512:(vh + 1) * 512],
                                              in_=acc[:, m, vh * 512:(vh + 1) * 512])
    nc.compile()
    return nc


_GRAPH_CACHE = {}


def _get_graph(Ldim=L, Ddim=D):
    key = (Ldim, Ddim)
    if key not in _GRAPH_CACHE:
        _GRAPH_CACHE[key] = build_graph(Ldim, Ddim)
    return _GRAPH_CACHE[key]


def kernel(x, z, Wq, bq, Wk, bk, Wv, bv):
    x = np.ascontiguousarray(np.asarray(x, dtype=np.float32)).astype(BF16_NP)
    z = np.ascontiguousarray(np.asarray(z, dtype=np.float32)).astype(BF16_NP)
    Ldim, Ddim = x.shape
    NPART = P
    nc = _get_graph(Ldim, Ddim)
    ROWS = Ldim // NCORES
    common = {
        "Wq": np.ascontiguousarray(np.asarray(Wq, np.float32).astype(BF16_NP)),
        "bq": np.ascontiguousarray(np.asarray(bq, np.float32)),
        "Wk": np.ascontiguousarray(np.asarray(Wk, np.float32).astype(BF16_NP)),
        "bk": np.ascontiguousarray(np.asarray(bk, np.float32)),
        "Wv": np.ascontiguousarray(np.asarray(Wv, np.float32).astype(BF16_NP)),
        "bv": np.ascontiguousarray(np.asarray(bv, np.float32)),
    }
    in_maps = []
    for c in range(NCORES):
        m = dict(common)
        xc = x[c::NCORES]                      # interleaved query rows
        zc = z[ROWS * c:ROWS * (c + 1)]        # contiguous key rows
        m["x"] = np.ascontiguousarray(
            xc.reshape(ROWS // NPART, NPART, Ddim).transpose(1, 0, 2))
        m["z"] = np.ascontiguousarray(
            zc.reshape(ROWS // NPART, NPART, Ddim).transpose(1, 0, 2))
        m["cval"] = np.array([c], dtype=np.float32)
        in_maps.append(m)
    try:
        res = run_bass_kernel_spmd(nc, in_maps, core_ids=list(range(NCORES)))
    except Exception:
        # transient NRT device hiccups have been observed; one retry
        res = run_bass_kernel_spmd(nc, in_maps, core_ids=list(range(NCORES)))
    out = np.empty((Ldim, Ddim), dtype=np.float32)
    for c in range(NCORES):
        out[c::NCORES] = res.results[c]["out"]
    return out


# revision 42
# speedup vs baseline: 1.3012x; 1.0060x over previous
"""Distributed Trainium2 Bass kernel: masked (upper-triangular) attention.

reference (L=4096, D=1024, fp32):
    Q = x @ Wq + bq ; K = z @ Wk + bk ; V = z @ Wv + bv
    S = Q @ K.T ; S[row > col] = -inf
    out = softmax(S / sqrt(D)) @ V

Strategy (8 NeuronCores, one TRN2 chip, SPMD):
  - Query rows dealt round-robin: core c owns rows {r : r % 8 == c}. This
    makes the causal (keep col >= row) footprint IDENTICAL on every core:
    query chunk m (128 local rows = global rows c+8*(128m..)) attends key
    tile t (512 keys) iff 2m <= t -> a uniform static 20-unit schedule that
    skips ~44% of the S/PV work with no per-core addressing.
  - K/V projections sharded over contiguous z blocks (512/core), AllGathered
    in bf16 into Shared-address-space DRAM (K^T as [d,keys], V natural).
  - S computed in [q, k] orientation (Q^T chunk stationary, K^T tile moving
    512-wide); exp on scalar engine emits row-sums via accum_out; P^T for
    the PV matmul obtained with PE transposes of the 128x128 es chunks.
  - Only the two near-diagonal tiles per chunk need masks: two constant
    [128,512] additive (-50) masks built once from an iota + core id.
  - Matmuls in bf16 with fp32 PSUM accumulation.
"""

import math

import ml_dtypes
import numpy as np

BF16_NP = ml_dtypes.bfloat16

import concourse.mybir as mybir
import concourse.tile as tile
from concourse import bacc
from concourse.bass_utils import run_bass_kernel_spmd

F32 = mybir.dt.float32
BF16 = mybir.dt.bfloat16
AF = mybir.ActivationFunctionType
OP = mybir.AluOpType
P = 128
NCORES = 8

L = 4096
D = 1024


def build_graph(Ldim=L, Ddim=D):
    nc = bacc.Bacc("TRN2", target_bir_lowering=False, debug=False, num_devices=NCORES)
    ROWS = Ldim // NCORES        # query rows per core
    MB = ROWS // P               # 128-row query chunks per core (4)
    ZB = ROWS // P               # z-shard 128-row blocks (4)
    SW = ROWS                    # key-tile width == z-shard width (512)
    JT = SW // P                 # 128-key subtiles per key tile (4)
    NT = NCORES                  # one key tile per shard
    IO = Ddim // P               # contraction chunks (8)
    AO = Ddim // P               # d_attn 128-blocks (8)
    VH = Ddim // 512             # 512-wide value column halves (2)
    scale = 1.0 / math.sqrt(Ddim)
    # units (t, m) with 2m <= t; unit index = UOFF[t] + m
    UCNT = [t // 2 + 1 for t in range(NT)]
    UOFF = [sum(UCNT[:t]) for t in range(NT)]
    NU = sum(UCNT)               # 20

    x_ext = nc.declare_dram_parameter("x", [P, MB, Ddim], BF16, isOutput=False)
    z_ext = nc.declare_dram_parameter("z", [P, ZB, Ddim], BF16, isOutput=False)
    wq_ext = nc.declare_dram_parameter("Wq", [Ddim, Ddim], BF16, isOutput=False)
    wk_ext = nc.declare_dram_parameter("Wk", [Ddim, Ddim], BF16, isOutput=False)
    wv_ext = nc.declare_dram_parameter("Wv", [Ddim, Ddim], BF16, isOutput=False)
    bq_ext = nc.declare_dram_parameter("bq", [Ddim], F32, isOutput=False)
    bk_ext = nc.declare_dram_parameter("bk", [Ddim], F32, isOutput=False)
    bv_ext = nc.declare_dram_parameter("bv", [Ddim], F32, isOutput=False)
    cval_ext = nc.declare_dram_parameter("cval", [1], F32, isOutput=False)
    out_ext = nc.declare_dram_parameter("out", [ROWS, Ddim], F32, isOutput=True)

    ident_d = nc.inline_tensor(np.eye(P, dtype=np.float32), name="ident_c")
    identb_d = nc.inline_tensor(np.eye(P, dtype=np.float32), name="identb_c")

    with tile.TileContext(nc) as tc:
        with tc.tile_pool(name="const", bufs=1) as constp, \
             tc.tile_pool(name="persist", bufs=1) as persist, \
             tc.tile_pool(name="dram", bufs=1, space="DRAM") as dram:
            identf = constp.tile([P, P], F32)
            nc.scalar.dma_start(out=identf[:], in_=identb_d.ap())
            identb = constp.tile([P, P], BF16)
            nc.vector.tensor_copy(identb[:], identf[:])
            bvb = constp.tile([P, Ddim], F32)
            nc.scalar.dma_start(out=bvb[:], in_=bv_ext[:].partition_broadcast(P))
            bqs = constp.tile([P, AO], F32)
            nc.scalar.dma_start(out=bqs[:], in_=bq_ext[:].rearrange("(ao p) -> p ao", p=P))
            bks = constp.tile([P, AO], F32)
            nc.scalar.dma_start(out=bks[:], in_=bk_ext[:].rearrange("(ao p) -> p ao", p=P))
            cvb = constp.tile([P, 1], F32)
            nc.scalar.dma_start(out=cvb[:], in_=cval_ext[:].partition_broadcast(P))

            QT = persist.tile([P, AO, ROWS], BF16)
            KH = 1                       # key splits (1: single K AllGather)
            KW = AO * (ROWS // KH)       # flat K width per partition per half
            VW = ZB * Ddim               # flat V width per partition
            kt_bds = [dram.tile([P, AO, ROWS // KH], BF16, name=f"kt_bd{h}")
                      for h in range(KH)]
            v_bds = [dram.tile([P, VW // VH], BF16, name=f"v_bd{vh}") for vh in range(VH)]
            kt_gds = [dram.tile([NCORES, P, AO, ROWS // KH], BF16, name=f"kt_gd{h}",
                                addr_space="Shared") for h in range(KH)]
            v_gds = [dram.tile([NCORES, P, VW // VH], BF16, name=f"v_gd{vh}",
                               addr_space="Shared") for vh in range(VH)]

            # additive pre-softmax masks for the two near-diagonal tiles of
            # each query chunk: with r = c + 8i + 1024m, keys k = 512t + f:
            #   t == 2m  : keep iff f - 8i - c >= 0        (maskA)
            #   t == 2m+1: keep iff f - 8i - c + 512 >= 0  (maskB)
            maskA = persist.tile([P, SW], F32)
            maskB = persist.tile([P, SW], F32)
            with tc.tile_pool(name="iop", bufs=1) as iop:
                iof = iop.tile([P, SW], F32)
                nc.gpsimd.iota(iof[:], pattern=[[1, SW]], base=0,
                               channel_multiplier=-8,
                               allow_small_or_imprecise_dtypes=True)
                tA = iop.tile([P, SW], F32)
                nc.vector.tensor_scalar(tA[:], iof[:], cvb[:], None, OP.subtract)
                mkA = iop.tile([P, SW], F32)
                nc.vector.tensor_scalar(mkA[:], tA[:], 0.0, None, OP.is_ge)
                nc.vector.tensor_scalar(maskA[:], mkA[:], 1.0, 50.0, OP.subtract, OP.mult)
                tB = iop.tile([P, SW], F32)
                nc.vector.tensor_scalar(tB[:], tA[:], 512.0, None, OP.add)
                mkB = iop.tile([P, SW], F32)
                nc.vector.tensor_scalar(mkB[:], tB[:], 0.0, None, OP.is_ge)
                nc.vector.tensor_scalar(maskB[:], mkB[:], 1.0, 50.0, OP.subtract, OP.mult)

            # ------- Phase 1+2: projections of own shards; K/V AllGathered -------
            # Inputs arrive pre-cast to bf16 from the host: weights DMA straight
            # into their SBUF tiles (no staging/cast), transposes run in bf16.
            with tc.tile_pool(name="inp", bufs=1) as inp, \
                 tc.tile_pool(name="wkv", bufs=1) as wp, \
                 tc.tile_pool(name="zp", bufs=1) as zp, \
                 tc.tile_pool(name="tpp", bufs=2, space="PSUM") as tpp, \
                 tc.tile_pool(name="pp", bufs=2, space="PSUM") as pp:
                zsb = inp.tile([P, ZB, Ddim], BF16)
                nc.sync.dma_start(out=zsb[:], in_=z_ext[:])
                xsb = inp.tile([P, MB, Ddim], BF16)
                nc.sync.dma_start(out=xsb[:], in_=x_ext[:])
                wk = wp.tile([P, IO, Ddim], BF16)
                wv = wp.tile([P, IO, Ddim], BF16)
                wq = wp.tile([P, IO, Ddim], BF16)
                nc.scalar.dma_start(out=wk[:], in_=wk_ext[:].rearrange("(io p) d -> p io d", p=P))
                zT = zp.tile([P, IO, ROWS], BF16)
                for io in range(IO):
                    for nb in range(ZB):
                        tp = tpp.tile([P, P], BF16, tag="tp", name=f"tp_{nb}_{io}")
                        nc.tensor.transpose(tp[:], zsb[:, nb, io * P:(io + 1) * P], identb[:])
                        nc.vector.tensor_copy(zT[:, io, nb * P:(nb + 1) * P], tp[:])

                KTs = inp.tile([P, AO, ROWS], BF16)
                for ao in range(AO):
                    kp = pp.tile([P, ROWS], F32, tag="kp", name=f"kp_{ao}")
                    for io in range(IO):
                        nc.tensor.matmul(kp[:], wk[:, io, ao * P:(ao + 1) * P], zT[:, io, :],
                                         start=(io == 0), stop=(io == IO - 1))
                    nc.vector.tensor_scalar(KTs[:, ao, :], kp[:], bks[:, ao:ao + 1], None, OP.add)
                KHW = ROWS // KH
                for h in range(KH):
                    nc.sync.dma_start(out=kt_bds[h][:], in_=KTs[:, :, h * KHW:(h + 1) * KHW])
                    nc.gpsimd.collective_compute(
                        "AllGather", OP.bypass, replica_groups=[list(range(NCORES))],
                        ins=[kt_bds[h][:].opt()], outs=[kt_gds[h][:].opt()])

                # V next: its AllGathers queue on the CC engine right behind K
                nc.scalar.dma_start(out=wv[:], in_=wv_ext[:].rearrange("(io p) d -> p io d", p=P))
                Vs = inp.tile([P, VH, ZB, 512], BF16)
                for nb in range(ZB):
                    vp = pp.tile([P, Ddim], F32, tag="vp", name=f"vp_{nb}", bufs=1)
                    for io in range(IO):
                        for vh in range(VH):
                            nc.tensor.matmul(vp[:, vh * 512:(vh + 1) * 512],
                                             zT[:, io, nb * P:(nb + 1) * P],
                                             wv[:, io, vh * 512:(vh + 1) * 512],
                                             start=(io == 0), stop=(io == IO - 1))
                    for vh in range(VH):
                        nc.vector.tensor_tensor(Vs[:, vh, nb, :], vp[:, vh * 512:(vh + 1) * 512],
                                                bvb[:, vh * 512:(vh + 1) * 512], OP.add)
                for vh in range(VH):
                    nc.sync.dma_start(out=v_bds[vh][:], in_=Vs[:, vh])
                    nc.gpsimd.collective_compute(
                        "AllGather", OP.bypass, replica_groups=[list(range(NCORES))],
                        ins=[v_bds[vh][:].opt()], outs=[v_gds[vh][:].opt()])

                # Q^T projection (overlaps the K/V AllGathers)
                nc.scalar.dma_start(out=wq[:], in_=wq_ext[:].rearrange("(io p) d -> p io d", p=P))
                xT = zp.tile([P, IO, ROWS], BF16)
                for io in range(IO):
                    for mb in range(MB):
                        tq = tpp.tile([P, P], BF16, tag="tp", name=f"tq_{mb}_{io}")
                        nc.tensor.transpose(tq[:], xsb[:, mb, io * P:(io + 1) * P], identb[:])
                        nc.vector.tensor_copy(xT[:, io, mb * P:(mb + 1) * P], tq[:])
                for ao in range(AO):
                    qp = pp.tile([P, ROWS], F32, tag="kp", name=f"qp_{ao}")
                    for io in range(IO):
                        nc.tensor.matmul(qp[:], wq[:, io, ao * P:(ao + 1) * P], xT[:, io, :],
                                         start=(io == 0), stop=(io == IO - 1))
                    # fold the softmax 1/sqrt(D) into Q^T
                    nc.vector.tensor_scalar(QT[:, ao, :], qp[:], bqs[:, ao:ao + 1], float(scale),
                                            OP.add, OP.mult)

            # ---------------- Phase 3: attention ----------------
            esT = persist.tile([P, NU, JT, P], BF16)     # P^T chunks for PV
            lacc = persist.tile([P, MB], F32)            # softmax denominators
            acc = persist.tile([P, MB, Ddim], F32)       # normalized output staging
            dmae = (nc.sync, nc.scalar)

            # S pass: S[q,k] = Q^T-chunk (stationary) x K^T half-tile (moving);
            # two sub-passes, one per gathered key-half so compute starts
            # right after the first K AllGather lands. Exp on scalar emits
            # row-sums via accum_out; PE transposes yield the P^T chunks for
            # PV, enqueued one unit behind so tensor never waits on the exp.
            KHW = SW // KH
            JH = JT // KH                # 128-key chunks per half (2)
            with tc.tile_pool(name="ktp", bufs=5) as ktp, \
                 tc.tile_pool(name="esp", bufs=4) as esp, \
                 tc.tile_pool(name="lpps", bufs=4) as lpps, \
                 tc.tile_pool(name="spp", bufs=3, space="PSUM") as spp, \
                 tc.tile_pool(name="tp2", bufs=2, space="PSUM") as tp2:
                pend = []

                def flush_pend():
                    for (pes, pu, ph) in pend:
                        for j in range(JH):
                            kc = ph * JH + j
                            tp = tp2.tile([P, P], BF16, tag="tp2", name=f"tp2_{pu}_{kc}")
                            nc.tensor.transpose(tp[:], pes[:, j * P:(j + 1) * P], identb[:])
                            if kc % 2 == 0:
                                nc.scalar.activation(esT[:, pu, kc, :], tp[:], AF.Copy)
                            else:
                                nc.vector.tensor_copy(esT[:, pu, kc, :], tp[:])
                    pend.clear()

                for h in range(KH):
                    for t in range(NT):
                        ktt = ktp.tile([P, AO, KHW], BF16, tag="ktt", name=f"ktt_{h}_{t}")
                        if t == 0:
                            # split the first tile's load so the S pass starts
                            # on ao-chunk 0 without waiting for the full tile
                            nc.sync.dma_start(out=ktt[:, 0:2, :], in_=kt_gds[h][t][:, 0:2, :])
                            nc.sync.dma_start(out=ktt[:, 2:AO, :], in_=kt_gds[h][t][:, 2:AO, :])
                        else:
                            nc.sync.dma_start(out=ktt[:], in_=kt_gds[h][t])
                        for m in range(t // 2 + 1):
                            u = UOFF[t] + m
                            sp = spp.tile([P, KHW], F32, tag="sp", name=f"sp_{u}_{h}")
                            for ao in range(AO):
                                nc.tensor.matmul(sp[:], QT[:, ao, m * P:(m + 1) * P],
                                                 ktt[:, ao, :], start=(ao == 0),
                                                 stop=(ao == AO - 1))
                            flush_pend()
                            if t == 2 * m:
                                nc.vector.tensor_tensor(sp[:], sp[:],
                                                        maskA[:, h * KHW:(h + 1) * KHW],
                                                        OP.add)
                            elif t == 2 * m + 1:
                                nc.vector.tensor_tensor(sp[:], sp[:],
                                                        maskB[:, h * KHW:(h + 1) * KHW],
                                                        OP.add)
                            es = esp.tile([P, KHW], BF16, tag="es", name=f"es_{u}_{h}")
                            lp = lpps.tile([P, 1], F32, tag="lp", name=f"lp_{u}_{h}")
                            nc.scalar.activation(es[:], sp[:], AF.Exp, accum_out=lp[:])
                            if t == 2 * m and h == 0:
                                nc.vector.tensor_copy(lacc[:, m:m + 1], lp[:])
                            else:
                                nc.vector.tensor_tensor(lacc[:, m:m + 1], lacc[:, m:m + 1],
                                                        lp[:], OP.add)
                            pend.append((es, u, h))
                flush_pend()

            # PV pass per value-half, tiles descending so the deepest chunks
            # start immediately after the S pass; psum per query chunk. Each
            # chunk is normalized (and on the second half, written out) as
            # soon as its accumulation stops, spreading the output DMAs.
            oview = out_ext[:].rearrange("(mb p) v -> p mb v", p=P)
            with tc.tile_pool(name="vtp", bufs=8) as vtp, \
                 tc.tile_pool(name="recp", bufs=1) as recp, \
                 tc.tile_pool(name="pvp", bufs=1, space="PSUM") as pvp:
                rec = recp.tile([P, MB], F32)
                nc.vector.reciprocal(rec[:], lacc[:])
                for vh in range(VH):
                    pvs = [pvp.tile([P, 512], F32, tag=f"pv{m}", name=f"pv{vh}_{m}")
                           for m in range(MB)]
                    for t in range(NT - 1, -1, -1):
                        vtt = vtp.tile([P, JT, 512], BF16, tag="vtt", name=f"vtt_{vh}_{t}")
                        nc.gpsimd.dma_start(out=vtt[:], in_=v_gds[vh][t])
                        for m in range(t // 2 + 1):
                            u = UOFF[t] + m
                            for kc in range(JT):
                                nc.tensor.matmul(pvs[m][:], esT[:, u, kc, :],
                                                 vtt[:, kc, :],
                                                 start=(t == NT - 1 and kc == 0),
                                                 stop=(t == 2 * m and kc == JT - 1))
                        if t % 2 == 0:
                            m = t // 2
                            nc.scalar.activation(acc[:, m, vh * 512:(vh + 1) * 512],
                                                 pvs[m][:], AF.Copy, scale=rec[:, m:m + 1])
                            nc.sync.dma_start(out=oview[:, m, vh * 512:(vh + 1) * 512],
                                              in_=acc[:, m, vh * 512:(vh + 1) * 512])
    nc.compile()
    return nc


_GRAPH_CACHE = {}


def _get_graph(Ldim=L, Ddim=D):
    key = (Ldim, Ddim)
    if key not in _GRAPH_CACHE:
        _GRAPH_CACHE[key] = build_graph(Ldim, Ddim)
    return _GRAPH_CACHE[key]


def kernel(x, z, Wq, bq, Wk, bk, Wv, bv):
    x = np.ascontiguousarray(np.asarray(x, dtype=np.float32)).astype(BF16_NP)
    z = np.ascontiguousarray(np.asarray(z, dtype=np.float32)).astype(BF16_NP)
    Ldim, Ddim = x.shape
    NPART = P
    nc = _get_graph(Ldim, Ddim)
    ROWS = Ldim // NCORES
    common = {
        "Wq": np.ascontiguousarray(np.asarray(Wq, np.float32).astype(BF16_NP)),
        "bq": np.ascontiguousarray(np.asarray(bq, np.float32)),
        "Wk": np.ascontiguousarray(np.asarray(Wk, np.float32).astype(BF16_NP)),
        "bk": np.ascontiguousarray(np.asarray(bk, np.float32)),
        "Wv": np.ascontiguousarray(np.asarray(Wv, np.float32).astype(BF16_NP)),
        "bv": np.ascontiguousarray(np.asarray(bv, np.float32)),
    }
    in_maps = []
    for c in range(NCORES):
        m = dict(common)
        xc = x[c::NCORES]                      # interleaved query rows
        zc = z[ROWS * c:ROWS * (c + 1)]        # contiguous key rows
        m["x"] = np.ascontiguousarray(
            xc.reshape(ROWS // NPART, NPART, Ddim).transpose(1, 0, 2))
        m["z"] = np.ascontiguousarray(
            zc.reshape(ROWS // NPART, NPART, Ddim).transpose(1, 0, 2))
        m["cval"] = np.array([c], dtype=np.float32)
        in_maps.append(m)
    try:
        res = run_bass_kernel_spmd(nc, in_maps, core_ids=list(range(NCORES)))
    except Exception:
        # transient NRT device hiccups have been observed; one retry
        res = run_bass_kernel_spmd(nc, in_maps, core_ids=list(range(NCORES)))
    out = np.empty((Ldim, Ddim), dtype=np.float32)
    for c in range(NCORES):
        out[c::NCORES] = res.results[c]["out"]
    return out


# revision 43
# speedup vs baseline: 1.3178x; 1.0128x over previous
"""Distributed Trainium2 Bass kernel: masked (upper-triangular) attention.

reference (L=4096, D=1024, fp32):
    Q = x @ Wq + bq ; K = z @ Wk + bk ; V = z @ Wv + bv
    S = Q @ K.T ; S[row > col] = -inf
    out = softmax(S / sqrt(D)) @ V

Strategy (8 NeuronCores, one TRN2 chip, SPMD):
  - Query rows dealt round-robin: core c owns rows {r : r % 8 == c}. This
    makes the causal (keep col >= row) footprint IDENTICAL on every core:
    query chunk m (128 local rows = global rows c+8*(128m..)) attends key
    tile t (512 keys) iff 2m <= t -> a uniform static 20-unit schedule that
    skips ~44% of the S/PV work with no per-core addressing.
  - K/V projections sharded over contiguous z blocks (512/core), AllGathered
    in bf16 into Shared-address-space DRAM (K^T as [d,keys], V natural).
  - S computed in [q, k] orientation (Q^T chunk stationary, K^T tile moving
    512-wide); exp on scalar engine emits row-sums via accum_out; P^T for
    the PV matmul obtained with PE transposes of the 128x128 es chunks.
  - Only the two near-diagonal tiles per chunk need masks: two constant
    [128,512] additive (-50) masks built once from an iota + core id.
  - Matmuls in bf16 with fp32 PSUM accumulation.
"""

import math

import ml_dtypes
import numpy as np

BF16_NP = ml_dtypes.bfloat16

import concourse.mybir as mybir
import concourse.tile as tile
from concourse import bacc
from concourse.bass_utils import run_bass_kernel_spmd

F32 = mybir.dt.float32
BF16 = mybir.dt.bfloat16
AF = mybir.ActivationFunctionType
OP = mybir.AluOpType
P = 128
NCORES = 8

L = 4096
D = 1024


def build_graph(Ldim=L, Ddim=D):
    nc = bacc.Bacc("TRN2", target_bir_lowering=False, debug=False, num_devices=NCORES)
    ROWS = Ldim // NCORES        # query rows per core
    MB = ROWS // P               # 128-row query chunks per core (4)
    ZB = ROWS // P               # z-shard 128-row blocks (4)
    SW = ROWS                    # key-tile width == z-shard width (512)
    JT = SW // P                 # 128-key subtiles per key tile (4)
    NT = NCORES                  # one key tile per shard
    IO = Ddim // P               # contraction chunks (8)
    AO = Ddim // P               # d_attn 128-blocks (8)
    VH = Ddim // 512             # 512-wide value column halves (2)
    scale = 1.0 / math.sqrt(Ddim)
    # units (t, m) with 2m <= t; unit index = UOFF[t] + m
    UCNT = [t // 2 + 1 for t in range(NT)]
    UOFF = [sum(UCNT[:t]) for t in range(NT)]
    NU = sum(UCNT)               # 20

    x_ext = nc.declare_dram_parameter("x", [P, MB, Ddim], BF16, isOutput=False)
    z_ext = nc.declare_dram_parameter("z", [P, ZB, Ddim], BF16, isOutput=False)
    wq_ext = nc.declare_dram_parameter("Wq", [Ddim, Ddim], BF16, isOutput=False)
    wk_ext = nc.declare_dram_parameter("Wk", [Ddim, Ddim], BF16, isOutput=False)
    wv_ext = nc.declare_dram_parameter("Wv", [Ddim, Ddim], BF16, isOutput=False)
    bq_ext = nc.declare_dram_parameter("bq", [Ddim], F32, isOutput=False)
    bk_ext = nc.declare_dram_parameter("bk", [Ddim], F32, isOutput=False)
    bv_ext = nc.declare_dram_parameter("bv", [Ddim], F32, isOutput=False)
    cval_ext = nc.declare_dram_parameter("cval", [1], F32, isOutput=False)
    out_ext = nc.declare_dram_parameter("out", [ROWS, Ddim], F32, isOutput=True)

    ident_d = nc.inline_tensor(np.eye(P, dtype=np.float32), name="ident_c")
    identb_d = nc.inline_tensor(np.eye(P, dtype=np.float32), name="identb_c")

    with tile.TileContext(nc) as tc:
        with tc.tile_pool(name="const", bufs=1) as constp, \
             tc.tile_pool(name="persist", bufs=1) as persist, \
             tc.tile_pool(name="dram", bufs=1, space="DRAM") as dram:
            identf = constp.tile([P, P], F32)
            nc.scalar.dma_start(out=identf[:], in_=identb_d.ap())
            identb = constp.tile([P, P], BF16)
            nc.vector.tensor_copy(identb[:], identf[:])
            bvb = constp.tile([P, Ddim], F32)
            nc.scalar.dma_start(out=bvb[:], in_=bv_ext[:].partition_broadcast(P))
            bqs = constp.tile([P, AO], F32)
            nc.scalar.dma_start(out=bqs[:], in_=bq_ext[:].rearrange("(ao p) -> p ao", p=P))
            bks = constp.tile([P, AO], F32)
            nc.scalar.dma_start(out=bks[:], in_=bk_ext[:].rearrange("(ao p) -> p ao", p=P))
            cvb = constp.tile([P, 1], F32)
            nc.scalar.dma_start(out=cvb[:], in_=cval_ext[:].partition_broadcast(P))

            QT = persist.tile([P, AO, ROWS], BF16)
            KH = 1                       # key splits (1: single K AllGather)
            KW = AO * (ROWS // KH)       # flat K width per partition per half
            VW = ZB * Ddim               # flat V width per partition
            kt_bds = [dram.tile([P, AO, ROWS // KH], BF16, name=f"kt_bd{h}")
                      for h in range(KH)]
            v_bds = [dram.tile([P, VW // VH], BF16, name=f"v_bd{vh}") for vh in range(VH)]
            kt_gds = [dram.tile([NCORES, P, AO, ROWS // KH], BF16, name=f"kt_gd{h}",
                                addr_space="Shared") for h in range(KH)]
            v_gds = [dram.tile([NCORES, P, VW // VH], BF16, name=f"v_gd{vh}",
                               addr_space="Shared") for vh in range(VH)]

            # additive pre-softmax masks for the two near-diagonal tiles of
            # each query chunk: with r = c + 8i + 1024m, keys k = 512t + f:
            #   t == 2m  : keep iff f - 8i - c >= 0        (maskA)
            #   t == 2m+1: keep iff f - 8i - c + 512 >= 0  (maskB)
            maskA = persist.tile([P, SW], F32)
            maskB = persist.tile([P, SW], F32)
            with tc.tile_pool(name="iop", bufs=1) as iop:
                iof = iop.tile([P, SW], F32)
                nc.gpsimd.iota(iof[:], pattern=[[1, SW]], base=0,
                               channel_multiplier=-8,
                               allow_small_or_imprecise_dtypes=True)
                tA = iop.tile([P, SW], F32)
                nc.vector.tensor_scalar(tA[:], iof[:], cvb[:], None, OP.subtract)
                mkA = iop.tile([P, SW], F32)
                nc.vector.tensor_scalar(mkA[:], tA[:], 0.0, None, OP.is_ge)
                nc.vector.tensor_scalar(maskA[:], mkA[:], 1.0, 50.0, OP.subtract, OP.mult)
                tB = iop.tile([P, SW], F32)
                nc.vector.tensor_scalar(tB[:], tA[:], 512.0, None, OP.add)
                mkB = iop.tile([P, SW], F32)
                nc.vector.tensor_scalar(mkB[:], tB[:], 0.0, None, OP.is_ge)
                nc.vector.tensor_scalar(maskB[:], mkB[:], 1.0, 50.0, OP.subtract, OP.mult)

            # ------- Phase 1+2: projections of own shards; K/V AllGathered -------
            # Inputs arrive pre-cast to bf16 from the host: weights DMA straight
            # into their SBUF tiles (no staging/cast), transposes run in bf16.
            with tc.tile_pool(name="inp", bufs=1) as inp, \
                 tc.tile_pool(name="wkv", bufs=1) as wp, \
                 tc.tile_pool(name="zp", bufs=1) as zp, \
                 tc.tile_pool(name="tpp", bufs=2, space="PSUM") as tpp, \
                 tc.tile_pool(name="pp", bufs=2, space="PSUM") as pp:
                zsb = inp.tile([P, ZB, Ddim], BF16)
                nc.sync.dma_start(out=zsb[:], in_=z_ext[:])
                xsb = inp.tile([P, MB, Ddim], BF16)
                nc.sync.dma_start(out=xsb[:], in_=x_ext[:])
                wk = wp.tile([P, IO, Ddim], BF16)
                wv = wp.tile([P, IO, Ddim], BF16)
                wq = wp.tile([P, IO, Ddim], BF16)
                nc.scalar.dma_start(out=wk[:], in_=wk_ext[:].rearrange("(io p) d -> p io d", p=P))
                zT = zp.tile([P, IO, ROWS], BF16)
                for io in range(IO):
                    for nb in range(ZB):
                        tp = tpp.tile([P, P], BF16, tag="tp", name=f"tp_{nb}_{io}")
                        nc.tensor.transpose(tp[:], zsb[:, nb, io * P:(io + 1) * P], identb[:])
                        nc.vector.tensor_copy(zT[:, io, nb * P:(nb + 1) * P], tp[:])

                KTs = inp.tile([P, AO, ROWS], BF16)
                for ao in range(AO):
                    kp = pp.tile([P, ROWS], F32, tag="kp", name=f"kp_{ao}")
                    for io in range(IO):
                        nc.tensor.matmul(kp[:], wk[:, io, ao * P:(ao + 1) * P], zT[:, io, :],
                                         start=(io == 0), stop=(io == IO - 1))
                    nc.vector.tensor_scalar(KTs[:, ao, :], kp[:], bks[:, ao:ao + 1], None, OP.add)
                KHW = ROWS // KH
                for h in range(KH):
                    nc.sync.dma_start(out=kt_bds[h][:], in_=KTs[:, :, h * KHW:(h + 1) * KHW])
                    nc.gpsimd.collective_compute(
                        "AllGather", OP.bypass, replica_groups=[list(range(NCORES))],
                        ins=[kt_bds[h][:].opt()], outs=[kt_gds[h][:].opt()])

                # V next: its AllGathers queue on the CC engine right behind K
                nc.scalar.dma_start(out=wv[:], in_=wv_ext[:].rearrange("(io p) d -> p io d", p=P))
                Vs = inp.tile([P, VH, ZB, 512], BF16)
                for nb in range(ZB):
                    vp = pp.tile([P, Ddim], F32, tag="vp", name=f"vp_{nb}", bufs=1)
                    for io in range(IO):
                        for vh in range(VH):
                            nc.tensor.matmul(vp[:, vh * 512:(vh + 1) * 512],
                                             zT[:, io, nb * P:(nb + 1) * P],
                                             wv[:, io, vh * 512:(vh + 1) * 512],
                                             start=(io == 0), stop=(io == IO - 1))
                    for vh in range(VH):
                        nc.vector.tensor_tensor(Vs[:, vh, nb, :], vp[:, vh * 512:(vh + 1) * 512],
                                                bvb[:, vh * 512:(vh + 1) * 512], OP.add)
                for vh in range(VH):
                    nc.sync.dma_start(out=v_bds[vh][:], in_=Vs[:, vh])
                    nc.gpsimd.collective_compute(
                        "AllGather", OP.bypass, replica_groups=[list(range(NCORES))],
                        ins=[v_bds[vh][:].opt()], outs=[v_gds[vh][:].opt()])

                # Q^T projection (overlaps the K/V AllGathers)
                nc.scalar.dma_start(out=wq[:], in_=wq_ext[:].rearrange("(io p) d -> p io d", p=P))
                xT = zp.tile([P, IO, ROWS], BF16)
                for io in range(IO):
                    for mb in range(MB):
                        tq = tpp.tile([P, P], BF16, tag="tp", name=f"tq_{mb}_{io}")
                        nc.tensor.transpose(tq[:], xsb[:, mb, io * P:(io + 1) * P], identb[:])
                        nc.vector.tensor_copy(xT[:, io, mb * P:(mb + 1) * P], tq[:])
                for ao in range(AO):
                    qp = pp.tile([P, ROWS], F32, tag="kp", name=f"qp_{ao}")
                    for io in range(IO):
                        nc.tensor.matmul(qp[:], wq[:, io, ao * P:(ao + 1) * P], xT[:, io, :],
                                         start=(io == 0), stop=(io == IO - 1))
                    # fold the softmax 1/sqrt(D) into Q^T
                    nc.vector.tensor_scalar(QT[:, ao, :], qp[:], bqs[:, ao:ao + 1], float(scale),
                                            OP.add, OP.mult)

            # ---------------- Phase 3: attention ----------------
            esT = persist.tile([P, NU, JT, P], BF16)     # P^T chunks for PV
            lacc = persist.tile([P, MB], F32)            # softmax denominators
            acc = persist.tile([P, MB, Ddim], F32)       # normalized output staging
            dmae = (nc.sync, nc.scalar)

            # S pass: S[q,k] = Q^T-chunk (stationary) x K^T half-tile (moving);
            # two sub-passes, one per gathered key-half so compute starts
            # right after the first K AllGather lands. Exp on scalar emits
            # row-sums via accum_out; PE transposes yield the P^T chunks for
            # PV, enqueued one unit behind so tensor never waits on the exp.
            KHW = SW // KH
            JH = JT // KH                # 128-key chunks per half (2)
            with tc.tile_pool(name="ktp", bufs=5) as ktp, \
                 tc.tile_pool(name="esp", bufs=4) as esp, \
                 tc.tile_pool(name="lpps", bufs=4) as lpps, \
                 tc.tile_pool(name="spp", bufs=3, space="PSUM") as spp, \
                 tc.tile_pool(name="tp2", bufs=2, space="PSUM") as tp2:
                pend = []

                def flush_pend():
                    for (pes, pu, ph) in pend:
                        for j in range(JH):
                            kc = ph * JH + j
                            tp = tp2.tile([P, P], BF16, tag="tp2", name=f"tp2_{pu}_{kc}")
                            nc.tensor.transpose(tp[:], pes[:, j * P:(j + 1) * P], identb[:])
                            if kc % 2 == 0:
                                nc.scalar.activation(esT[:, pu, kc, :], tp[:], AF.Copy)
                            else:
                                nc.vector.tensor_copy(esT[:, pu, kc, :], tp[:])
                    pend.clear()

                for h in range(KH):
                    for t in range(NT - 1, -1, -1):
                        ktt = ktp.tile([P, AO, KHW], BF16, tag="ktt", name=f"ktt_{h}_{t}")
                        if t == NT - 1:
                            # split the first tile's load so the S pass starts
                            # on ao-chunk 0 without waiting for the full tile
                            nc.sync.dma_start(out=ktt[:, 0:2, :], in_=kt_gds[h][t][:, 0:2, :])
                            nc.sync.dma_start(out=ktt[:, 2:AO, :], in_=kt_gds[h][t][:, 2:AO, :])
                        else:
                            nc.sync.dma_start(out=ktt[:], in_=kt_gds[h][t])
                        for m in range(t // 2 + 1):
                            u = UOFF[t] + m
                            sp = spp.tile([P, KHW], F32, tag="sp", name=f"sp_{u}_{h}")
                            for ao in range(AO):
                                nc.tensor.matmul(sp[:], QT[:, ao, m * P:(m + 1) * P],
                                                 ktt[:, ao, :], start=(ao == 0),
                                                 stop=(ao == AO - 1))
                            flush_pend()
                            if t == 2 * m:
                                nc.vector.tensor_tensor(sp[:], sp[:],
                                                        maskA[:, h * KHW:(h + 1) * KHW],
                                                        OP.add)
                            elif t == 2 * m + 1:
                                nc.vector.tensor_tensor(sp[:], sp[:],
                                                        maskB[:, h * KHW:(h + 1) * KHW],
                                                        OP.add)
                            es = esp.tile([P, KHW], BF16, tag="es", name=f"es_{u}_{h}")
                            lp = lpps.tile([P, 1], F32, tag="lp", name=f"lp_{u}_{h}")
                            nc.scalar.activation(es[:], sp[:], AF.Exp, accum_out=lp[:])
                            if t == NT - 1 and h == 0:
                                nc.vector.tensor_copy(lacc[:, m:m + 1], lp[:])
                            else:
                                nc.vector.tensor_tensor(lacc[:, m:m + 1], lacc[:, m:m + 1],
                                                        lp[:], OP.add)
                            pend.append((es, u, h))
                flush_pend()

            # PV pass per value-half, tiles descending so the deepest chunks
            # start immediately after the S pass; psum per query chunk. Each
            # chunk is normalized (and on the second half, written out) as
            # soon as its accumulation stops, spreading the output DMAs.
            oview = out_ext[:].rearrange("(mb p) v -> p mb v", p=P)
            with tc.tile_pool(name="vtp", bufs=8) as vtp, \
                 tc.tile_pool(name="recp", bufs=1) as recp, \
                 tc.tile_pool(name="pvp", bufs=1, space="PSUM") as pvp:
                rec = recp.tile([P, MB], F32)
                nc.vector.reciprocal(rec[:], lacc[:])
                for vh in range(VH):
                    pvs = [pvp.tile([P, 512], F32, tag=f"pv{m}", name=f"pv{vh}_{m}")
                           for m in range(MB)]
                    for t in range(NT - 1, -1, -1):
                        vtt = vtp.tile([P, JT, 512], BF16, tag="vtt", name=f"vtt_{vh}_{t}")
                        nc.gpsimd.dma_start(out=vtt[:], in_=v_gds[vh][t])
                        for m in range(t // 2 + 1):
                            u = UOFF[t] + m
                            for kc in range(JT):
                                nc.tensor.matmul(pvs[m][:], esT[:, u, kc, :],
                                                 vtt[:, kc, :],
                                                 start=(t == NT - 1 and kc == 0),
                                                 stop=(t == 2 * m and kc == JT - 1))
                        if t % 2 == 0:
                            m = t // 2
                            nc.scalar.activation(acc[:, m, vh * 512:(vh + 1) * 512],
                                                 pvs[m][:], AF.Copy, scale=rec[:, m:m + 1])
                            nc.sync.dma_start(out=oview[:, m, vh * 512:(vh + 1) * 512],
                                              in_=acc[:, m, vh * 512:(vh + 1) * 512])
    nc.compile()
    return nc


_GRAPH_CACHE = {}


def _get_graph(Ldim=L, Ddim=D):
    key = (Ldim, Ddim)
    if key not in _GRAPH_CACHE:
        _GRAPH_CACHE[key] = build_graph(Ldim, Ddim)
    return _GRAPH_CACHE[key]


def kernel(x, z, Wq, bq, Wk, bk, Wv, bv):
    x = np.ascontiguousarray(np.asarray(x, dtype=np.float32)).astype(BF16_NP)
    z = np.ascontiguousarray(np.asarray(z, dtype=np.float32)).astype(BF16_NP)
    Ldim, Ddim = x.shape
    NPART = P
    nc = _get_graph(Ldim, Ddim)
    ROWS = Ldim // NCORES
    common = {
        "Wq": np.ascontiguousarray(np.asarray(Wq, np.float32).astype(BF16_NP)),
        "bq": np.ascontiguousarray(np.asarray(bq, np.float32)),
        "Wk": np.ascontiguousarray(np.asarray(Wk, np.float32).astype(BF16_NP)),
        "bk": np.ascontiguousarray(np.asarray(bk, np.float32)),
        "Wv": np.ascontiguousarray(np.asarray(Wv, np.float32).astype(BF16_NP)),
        "bv": np.ascontiguousarray(np.asarray(bv, np.float32)),
    }
    in_maps = []
    for c in range(NCORES):
        m = dict(common)
        xc = x[c::NCORES]                      # interleaved query rows
        zc = z[ROWS * c:ROWS * (c + 1)]        # contiguous key rows
        m["x"] = np.ascontiguousarray(
            xc.reshape(ROWS // NPART, NPART, Ddim).transpose(1, 0, 2))
        m["z"] = np.ascontiguousarray(
            zc.reshape(ROWS // NPART, NPART, Ddim).transpose(1, 0, 2))
        m["cval"] = np.array([c], dtype=np.float32)
        in_maps.append(m)
    try:
        res = run_bass_kernel_spmd(nc, in_maps, core_ids=list(range(NCORES)))
    except Exception:
        # transient NRT device hiccups have been observed; one retry
        res = run_bass_kernel_spmd(nc, in_maps, core_ids=list(range(NCORES)))
    out = np.empty((Ldim, Ddim), dtype=np.float32)
    for c in range(NCORES):
        out[c::NCORES] = res.results[c]["out"]
    return out
